# revision 30
# baseline (speedup 1.0000x reference)
"""Trainium2 Bass kernel for nn_Cy2Mixer_layer (gMLP block with conv/GCN/GCN
spatial mixers + fused output projection).

Sharding (8 cores):
  - The two GCN branches (sgu, cgu) + final projection/bias/residual are
    data-parallel over (B*T): 256 token-groups -> 32 per core, each a
    [N=128, D=256] tile (tokens on partitions).
  - The conv branch (tgu, Conv2d(T,T,(1,3)) channel mixer) needs full T per
    (b, n), so it is data-parallel over (B*N): 512 rows -> 64 per core,
    processed as 32 tiles of 2 rows ([2*T=128, D=256], tokens on partitions;
    the T-channel mix is a block-diagonal [128,128] matmul).
  Core outputs: og = xs/xc projections + b_out + residual (bt-sharded) and
  oc = xt projection (bn-sharded); the host scatters and adds the two.

v2: the two large-contraction matmul groups (w1: K=256, gcn-lin: K=512) run
as fp8e4m3 DoubleRow matmuls (2 stacked K-tiles per pass). Weights carry a
power-of-2 scale chosen on the host so 0.02-magnitude weights sit in fp8's
normal range; the scale is removed for free downstream (gelu's input scale
for w1, a pre-divided w2 for the gcn-lin). Bias matmuls whose vectors are
exactly zero for the given inputs are not emitted. Transposes run on the PE
(identity matmul) with GpSimd PSUM-evacuation instead of the DMA XBAR,
freeing the sync queue; LN applies/residual adds are spread across
DVE/GpSimd to balance the elementwise load.
"""

import os
import sys
from contextlib import ExitStack

for _p in ("/opt/trn_rl_repo", "/root/.axon_site/_ro/trn_rl_repo"):
    if os.path.isdir(_p) and _p not in sys.path:
        sys.path.insert(0, _p)

import numpy as np

import bass_rust
import concourse.bass as bass
import concourse.tile as tile
from concourse import mybir
from concourse.bass_utils import run_bass_kernel_spmd

if os.environ.get("LDW_OPT") == "1":
    from concourse import bass_utils as _bu
    _orig_run_command = _bu.run_command

    def _run_command_ldw(cmd, **kw):
        cmd = ["--enable-ldw-opt=true" if c == "--enable-ldw-opt=false" else c
               for c in cmd]
        return _orig_run_command(cmd, **kw)

    _bu.run_command = _run_command_ldw

AF = mybir.ActivationFunctionType
ALU = mybir.AluOpType
F32 = mybir.dt.float32
BF16 = mybir.dt.bfloat16
FP8 = mybir.dt.float8e4
LN_EPS = 1e-5

B, T, N, D, F = 4, 64, 128, 256, 512
NCORES = 8
N_GCN = 32   # bt tiles per core
N_CONV = 32  # conv tiles per core (2 bn rows each)

W1_SCALE = 64.0   # folded out via gelu's input scale
GR_SCALE = 8.0    # folded out via pre-divided w2
FIN_SCALE = 4.0   # split scale: xsh carries 1/4 (via rstd), wpr carries x4

_ctr = [0]


def _split_multi_waits(nc):
    """This walrus build rejects any instruction carrying >1 sync wait
    ("Too many sync wait commands"). Hoist all-but-one wait of every
    instruction onto dedicated same-engine NOPs inserted before it."""
    for f in nc.m.functions:
        for bb in f.blocks:
            insts = bb.instructions
            i = 0
            while i < len(insts):
                inst = insts[i]
                si = inst.sync_info
                if si is not None and si.on_wait is not None and len(si.on_wait) > 1:
                    waits = list(si.on_wait)
                    upd = list(si.on_update) if si.on_update is not None else []
                    inst.sync_info = bass_rust.SyncInfo(
                        on_wait=[waits[-1]], on_update=upd
                    )
                    for w in waits[:-1]:
                        _ctr[0] += 1
                        nop = mybir.InstNoOp(
                            name=f"wsplit-{_ctr[0]}",
                            sync_info=mybir.SyncInfo(on_wait=[w], on_update=[]),
                            bass_nofuse=True,
                            engine=inst.engine,
                        )
                        insts.insert(i, nop)
                        i += 1
                i += 1


def _fp8(x):
    dt = np.dtype(mybir.dt.np(FP8))
    return np.clip(np.asarray(x, np.float32), -240.0, 240.0).astype(dt)


def _dr_pack(w):
    """[K=256 or 512, Fout] -> fp8 DoubleRow layout [128, J, 2, Fout] flattened
    to [128, J*2*Fout] where pair j covers K-chunks (2j, 2j+1)."""
    K, Fo = w.shape
    nj = K // 256
    out = np.zeros((128, nj, 2, Fo), np.float32)
    for j in range(nj):
        for k in range(2):
            out[:, j, k, :] = w[(2 * j + k) * 128 : (2 * j + k + 1) * 128, :]
    return out.reshape(128, nj * 2 * Fo)


def _dr_pack_w1(w1p):
    """[D=256, 2F=1024] -> [128, 2048] with layout [p, (half, k, f512)]:
    half j selects the output 512-block, k the D-chunk of the contraction."""
    return np.concatenate(
        [_dr_pack(w1p[:, 0:512]), _dr_pack(w1p[:, 512:1024])], axis=1
    )


def _host_prep(inp):
    """Fold LN affines into weights; build matmul-ready constant layouts."""
    f32 = np.float32
    bf = np.dtype(mybir.dt.np(BF16))
    c = {}
    flags = {}
    cir = np.asarray(inp["cirmat"])
    a = (cir != 0).astype(f32)
    np.fill_diagonal(a, 1.0)
    deg = a.sum(0).astype(f32)
    dinv = (1.0 / np.sqrt(deg)).astype(f32)
    a_hat = (a * dinv[:, None] * dinv[None, :]).astype(f32)
    c["a_hat"] = a_hat
    colsum = a_hat.sum(0).astype(f32)

    c["ident"] = np.eye(128, dtype=f32)
    c["eps_col"] = np.full((128, 1), LN_EPS, f32)
    c["eps_col_f"] = np.full((128, 1), LN_EPS * FIN_SCALE * FIN_SCALE, f32)
    c["ones_row"] = np.ones((1, 128), f32)

    w_out = np.asarray(inp["w_out"])
    bout = np.asarray(inp["b_out"]).astype(f32).copy()

    fp8_out = {}

    for p, pre, ng_, nb_, off in (
        ("s", "sgu", "n2_g", "n2_b", 256),
        ("c", "cgu", "n3_g", "n3_b", 512),
    ):
        ng = np.asarray(inp[f"{pre}_ng"])
        nb = np.asarray(inp[f"{pre}_nb"])
        w1 = np.asarray(inp[f"{pre}_w1"])
        b1 = np.asarray(inp[f"{pre}_b1"])
        sg = np.asarray(inp[f"{pre}_sg"])
        sb = np.asarray(inp[f"{pre}_sb"])
        gw = np.asarray(inp[f"{pre}_gw"])
        gb = np.asarray(inp[f"{pre}_gb"])
        w2 = np.asarray(inp[f"{pre}_w2"])
        b2 = np.asarray(inp[f"{pre}_b2"])
        w1p = np.ascontiguousarray((w1 * ng[None, :]).T).astype(f32)  # [D, 2F]
        fp8_out[f"{p}_w18"] = _dr_pack_w1(w1p * W1_SCALE)
        b1f = (b1 + w1 @ nb).astype(f32)
        flags[f"{p}_b1"] = bool(np.any(b1f))
        c[f"{p}_b1"] = (b1f * W1_SCALE)[None, :]
        grhs = np.ascontiguousarray((gw * sg[None, :]).T).astype(f32)  # [F, F]
        fp8_out[f"{p}_grhs8"] = _dr_pack(grhs * GR_SCALE)
        c[f"{p}_bg_l"] = np.stack([colsum, np.ones(128, f32)]).astype(f32)
        c[f"{p}_bg_r"] = (np.stack([gw @ sb, gb]) * GR_SCALE).astype(f32)
        c[f"{p}_w2r"] = np.ascontiguousarray(w2.T).astype(f32) / GR_SCALE
        flags[f"{p}_b2"] = bool(np.any(b2))
        c[f"{p}_b2"] = b2[None, :].astype(f32)
        wsl = w_out[:, off : off + 256]
        c[f"{p}_wpr"] = np.ascontiguousarray((wsl * np.asarray(inp[ng_])[None, :]).T).astype(f32)
        bout = bout + wsl @ np.asarray(inp[nb_])

    ng = np.asarray(inp["tgu_ng"])
    nb = np.asarray(inp["tgu_nb"])
    w1 = np.asarray(inp["tgu_w1"])
    b1 = np.asarray(inp["tgu_b1"])
    sg = np.asarray(inp["tgu_sg"])
    sb = np.asarray(inp["tgu_sb"])
    cw = np.asarray(inp["tgu_cw"])[:, :, 0, :]  # [to, ti, dx]
    cb = np.asarray(inp["tgu_cb"])
    w2 = np.asarray(inp["tgu_w2"])
    b2 = np.asarray(inp["tgu_b2"])
    w1p = np.ascontiguousarray((w1 * ng[None, :]).T).astype(f32)
    fp8_out["t_w18"] = _dr_pack_w1(w1p * W1_SCALE)
    b1f = (b1 + w1 @ nb).astype(f32)
    flags["t_b1"] = bool(np.any(b1f))
    c["t_b1"] = (b1f * W1_SCALE)[None, :]
    for dx in range(3):
        blk = np.zeros((128, 128), f32)
        lh = np.ascontiguousarray(cw[:, :, dx].T)  # [ti, to]
        blk[:64, :64] = lh
        blk[64:, 64:] = lh
        c[f"t_cw{dx}"] = blk
    # conv bias as a rank-4 matmul: cb + sum_dx cwsum_dx[to]*sb[fo+dx-1]
    cwsum = cw.sum(1)  # [to, dx]
    lhs = np.zeros((4, 128), f32)
    lhs[0] = np.concatenate([cb, cb])
    for dx in range(3):
        lhs[1 + dx] = np.concatenate([cwsum[:, dx], cwsum[:, dx]])
    rhs = np.zeros((4, 512), f32)
    rhs[0] = 1.0
    rhs[1, 1:] = sb[:511]   # dx=0 reads sb[fo-1]
    rhs[2] = sb             # dx=1 reads sb[fo]
    rhs[3, :511] = sb[1:]   # dx=2 reads sb[fo+1]
    c["t_cb_l"] = lhs
    c["t_cb_r"] = rhs
    c["t_w2r"] = np.ascontiguousarray(w2.T).astype(f32)
    flags["t_b2"] = bool(np.any(b2))
    c["t_b2"] = b2[None, :].astype(f32)
    wsl = w_out[:, 0:256]
    c["t_wpr"] = np.ascontiguousarray((wsl * np.asarray(inp["n1_g"])[None, :]).T).astype(f32)
    bout = bout + wsl @ np.asarray(inp["n1_b"])
    flags["t_sg"] = bool(np.any(sg != 1.0))
    c["t_sg"] = np.broadcast_to(sg, (128, 512)).astype(f32).copy()
    flags["bout"] = bool(np.any(bout))
    c["bout"] = bout[None, :].astype(f32)

    out = {}
    for k, v in c.items():
        if k.startswith("eps_col"):
            out[k] = np.ascontiguousarray(v, dtype=f32)
        else:
            out[k] = np.ascontiguousarray(v).astype(bf)
    for k, v in fp8_out.items():
        out[k] = _fp8(v)
    return out, flags


# Constant tensors DMA'd to SBUF once.
_WSHAPES = {
    "a_hat": ([128, 128], BF16), "ident": ([128, 128], BF16),
    "ones_row": ([1, 128], BF16), "eps_col": ([128, 1], F32),
    "eps_col_f": ([128, 1], F32),
    "t_w18": ([128, 2048], FP8), "t_b1": ([1, 1024], BF16),
    "t_cw0": ([128, 128], BF16), "t_cw1": ([128, 128], BF16),
    "t_cw2": ([128, 128], BF16),
    "t_cb_l": ([4, 128], BF16), "t_cb_r": ([4, 512], BF16),
    "t_w2r": ([512, 256], BF16), "t_b2": ([1, 256], BF16),
    "t_wpr": ([256, 256], BF16),
    "t_sg": ([128, 512], BF16), "bout": ([1, 256], BF16),
}
for _p in ("s", "c"):
    _WSHAPES.update({
        f"{_p}_w18": ([128, 2048], FP8), f"{_p}_b1": ([1, 1024], BF16),
        f"{_p}_grhs8": ([128, 2048], FP8),
        f"{_p}_bg_l": ([2, 128], BF16), f"{_p}_bg_r": ([2, 512], BF16),
        f"{_p}_w2r": ([512, 256], BF16), f"{_p}_b2": ([1, 256], BF16),
        f"{_p}_wpr": ([256, 256], BF16),
    })

DR = mybir.MatmulPerfMode.DoubleRow


def _emit(nc, n_gcn, n_conv, flags):
    xg = nc.dram_tensor("xg", [n_gcn * 128, 256], F32, kind="ExternalInput")
    xc = nc.dram_tensor("xc", [n_conv * 128, 256], F32, kind="ExternalInput")
    og = nc.dram_tensor("og", [n_gcn * 128, 256], F32, kind="ExternalOutput")
    oc = nc.dram_tensor("oc", [n_conv * 128, 256], F32, kind="ExternalOutput")

    wd = {
        k: nc.dram_tensor(k, shp, dt, kind="ExternalInput")
        for k, (shp, dt) in _WSHAPES.items()
    }

    with tile.TileContext(nc) as tc, ExitStack() as ctx:
        cpool = ctx.enter_context(tc.tile_pool(name="consts", bufs=1))
        wp = ctx.enter_context(tc.tile_pool(name="work", bufs=int(os.environ.get("WP_BUFS", "4"))))
        wpl = ctx.enter_context(tc.tile_pool(name="workl", bufs=int(os.environ.get("WPL_BUFS", "10"))))
        sp = ctx.enter_context(tc.tile_pool(name="stats", bufs=int(os.environ.get("SP_BUFS", "8"))))
        pp = ctx.enter_context(tc.tile_pool(name="ps", bufs=int(os.environ.get("FIN_BUFS", "1")), space="PSUM"))
        ppa = ctx.enter_context(tc.tile_pool(name="psa", bufs=int(os.environ.get("GW_BUFS", "2")), space="PSUM"))
        ppt = ctx.enter_context(tc.tile_pool(name="pst", bufs=int(os.environ.get("TP_BUFS", "2")), space="PSUM"))
        pps = ctx.enter_context(tc.tile_pool(name="pss", bufs=int(os.environ.get("PSS_BUFS", "1")), space="PSUM"))
        pph = ctx.enter_context(tc.tile_pool(name="psh", bufs=int(os.environ.get("HP_BUFS", "1")), space="PSUM"))

        GRP = int(os.environ.get("GRP", "8"))
        SC_APPLY = os.environ.get("SC_APPLY", "0") == "1"

        # Const loads: order by first use; round-robin dispatch queues.
        _prio = ["eps_col", "ident", "ones_row", "s_w18", "s_b1", "c_w18",
                 "c_b1", "a_hat", "s_grhs8", "s_bg_l", "s_bg_r", "s_w2r",
                 "s_b2", "c_grhs8", "c_bg_l", "c_bg_r", "c_w2r", "c_b2",
                 "s_wpr", "c_wpr", "bout", "t_w18", "t_b1", "t_sg", "t_cw0",
                 "t_cw1", "t_cw2", "t_cb_l", "t_cb_r", "t_w2r", "t_b2",
                 "t_wpr"]
        _qs = [nc.sync, nc.scalar, nc.gpsimd]
        _qi = [0]

        def _cdma(dst, src):
            _qs[_qi[0] % len(_qs)].dma_start(dst, src)
            _qi[0] += 1

        CS = {}

        def _load_const(name):
            d = wd[name]
            dt = d.dtype
            pdim = d.shape[0]
            if pdim <= 128:
                t = cpool.tile(list(d.shape), dt, tag=name)
                _cdma(t[:], d[:])
                CS[name] = t
            else:
                ts = []
                for i in range(pdim // 128):
                    t = cpool.tile([128, d.shape[1]], dt, tag=f"{name}{i}")
                    _cdma(t[:], d[i * 128 : (i + 1) * 128, :])
                    ts.append(t)
                CS[name] = ts

        _load_const("eps_col")
        _load_const("eps_col_f")
        _load_const("ident")

        def ln_pre(xin, vc, i, tag):
            """bn_stats + bn_aggr into columns [2i, 2i+1] of the group's
            shared (mean, var) tile vc."""
            s6 = sp.tile([128, 6], F32, tag=f"s6{tag}")
            nc.vector.bn_stats(s6[:], xin)
            nc.vector.bn_aggr(vc[:, 2 * i : 2 * i + 2], s6[:])

        def ln_finish(vc, k, tag, fin_scaled=False):
            """One sqrt + one reciprocal over the whole group's stat tile.
            Even columns hold means (sqrt/recip of those are garbage but
            never read); odd columns become rstd. With fin_scaled, computes
            sqrt(S^2 var + S^2 eps) = S*std so rc = rstd/S (S=FIN_SCALE) --
            the 1/S that pairs with the x S baked into the fp8 wpr."""
            sc = sp.tile([128, 2 * k], F32, tag=f"sc{tag}")
            if fin_scaled:
                _chain(nc.scalar.activation(
                    sc[:], vc[:, 0 : 2 * k], AF.Sqrt,
                    bias=CS["eps_col_f"][:], scale=FIN_SCALE * FIN_SCALE))
            else:
                _chain(nc.scalar.activation(sc[:], vc[:, 0 : 2 * k], AF.Sqrt,
                                            bias=CS["eps_col"][:]))
            rc = sp.tile([128, 2 * k], F32, tag=f"rc{tag}")
            nc.vector.reciprocal(rc[:], sc[:])
            if not SC_APPLY:
                return rc, None
            vc3 = vc[:, 0 : 2 * k].rearrange("p (k two) -> p k two", two=2)
            rc3 = rc[:, 0 : 2 * k].rearrange("p (k two) -> p k two", two=2)
            mr = sp.tile([128, k], F32, tag=f"mr{tag}")
            nc.vector.tensor_tensor(mr[:], vc3[:, :, 0:1], rc3[:, :, 1:2],
                                    op=ALU.mult)
            nmr = sp.tile([128, k], F32, tag=f"nmr{tag}")
            nc.vector.tensor_scalar(nmr[:], mr[:], scalar1=-1.0, scalar2=None,
                                    op0=ALU.mult)
            return rc, nmr

        def ln_apply(xin, width, vc, rc, i, tag, eng=None, nmr=None):
            """bf16 (x - mean) * rstd."""
            pool = wpl if tag.startswith(("ln2", "ln3")) else wp
            out = pool.tile([128, width], BF16, tag=f"nrm{tag}", bufs=GRP + 2)
            if nmr is not None:
                # scalar engine: x*rstd + (-mean*rstd)
                nc.scalar.activation(
                    out[:], xin, AF.Identity,
                    bias=nmr[:, i : i + 1],
                    scale=rc[:, 2 * i + 1 : 2 * i + 2],
                )
            else:
                (eng or nc.vector).tensor_scalar(
                    out[:], xin, scalar1=vc[:, 2 * i : 2 * i + 1],
                    scalar2=rc[:, 2 * i + 1 : 2 * i + 2],
                    op0=ALU.subtract, op1=ALU.mult,
                )
            return out

        def pe_transpose(xin, width, tag, out_dt=BF16, evac=None):
            """PE identity transpose + engine evacuation (cast on write).
            GpSimd can't touch PSUM, so evac is scalar (default) or DVE."""
            out = (wpl if tag == "xhT" else wp).tile(
                [128, width], out_dt, tag=tag,
                bufs=(GRP + 2) if tag == "xhT" else None)
            tps = ppt.tile([128, 512], BF16, tag="tpose")
            for cc in range(width // 128):
                sl = slice(cc * 128, (cc + 1) * 128)
                nc.tensor.transpose(tps[:, sl], xin[:, sl], CS["ident"][:])
            (evac or nc.scalar.copy)(out[:], tps[:, 0:width])
            return out

        def dma_transpose(xin, width, tag, q="sync"):
            out = (wpl if tag.startswith("xhT") else wp).tile(
                [128, width], BF16, tag=tag,
                bufs=(GRP + 2) if tag.startswith("xhT") else None)
            o3 = out[:].rearrange("p (c q) -> p c q", q=128)
            getattr(nc, q).dma_start_transpose(o3, xin[:])
            return out

        def mlp_in(xhT8, w18, b1, has_b1, tag):
            """fp8 DoubleRow x̂ @ w1p (x W1_SCALE) [+ b1] into PSUM."""
            h_ps = pph.tile([128, 1024], F32, tag="hps")
            lhs3 = xhT8[:].rearrange("p (k m) -> p k m", k=2)
            for j in range(2):
                hj = h_ps[:, j * 512 : (j + 1) * 512]
                w3 = w18[:, j * 1024 : (j + 1) * 1024].rearrange(
                    "p (k f) -> p k f", k=2)
                nc.tensor.matmul(
                    hj, lhs3, w3, start=True, stop=not has_b1,
                    perf_mode=DR,
                )
                if has_b1:
                    nc.tensor.matmul(
                        hj, CS["ones_row"][:], b1[:, j * 512 : (j + 1) * 512],
                        start=False, stop=True, skip_group_check=True,
                    )
            return h_ps

        def gelu_of(h_ps, tag):
            h = wpl.tile([128, 1024], BF16, tag=tag, bufs=GRP + 2)
            _chain(nc.scalar.activation(h[:], h_ps[:], AF.Gelu, scale=1.0 / W1_SCALE))
            return h

        GT_DMA = os.environ.get("GT_DMA", "1") == "1"

        def backend_mid(gated_src, u, X, p):
            """gated = psum*u, transpose, w2 matmul [+ b2] + residual -> blk."""
            gated = wp.tile([128, 512], BF16, tag="gated")
            nc.vector.tensor_tensor(gated[:], gated_src[:], u, op=ALU.mult)
            if GT_DMA:
                gT = dma_transpose(gated[:], 512, "gT")
            else:
                gT = pe_transpose(gated[:], 512, "gT")
            blk_ps = pps.tile([128, 256], F32, tag="smallps")
            has_b2 = flags[f"{p}_b2"]
            for fc in range(4):
                sl = slice(fc * 128, (fc + 1) * 128)
                nc.tensor.matmul(
                    blk_ps[:], gT[:, sl], CS[f"{p}_w2r"][fc][:],
                    start=(fc == 0), stop=(fc == 3 and not has_b2),
                )
            if has_b2:
                nc.tensor.matmul(
                    blk_ps[:], CS["ones_row"][:], CS[f"{p}_b2"][:],
                    start=False, stop=True, skip_group_check=True,
                )
            blk = wpl.tile([128, 256], F32, tag=f"blk{p}", bufs=GRP + 1)
            nc.vector.tensor_tensor(blk[:], blk_ps[:], X[:], op=ALU.add)
            return blk

        # ---------------- software-pipelined tile stream ----------------
        # Tiles 0..n_gcn-1 are GCN (xg), the rest conv (xc). Processed in
        # groups of GRP with phase A (load+LN1+transpose) of group g emitted
        # before phases B (mlp+gelu) / C (mixers+backends) of group g-1, so
        # the scalar engine sees [sqrt block][gelu block] per group — 2
        # act-table trips per group instead of 2+ per tile. The _chain dep
        # forces that issue order (the tile scheduler is a greedy ready-heap
        # and would otherwise interleave).
        _last_act = [None]

        def _chain(bi):
            if _last_act[0] is not None:
                tile.add_dep_helper(bi.ins, _last_act[0].ins, reason="act-table order")
            _last_act[0] = bi
            return bi

        XHT_DMA = os.environ.get("XHT_DMA", "0") == "1"

        def phase_a_group(ts):
            states = []
            vc = sp.tile([128, 2 * len(ts)], F32, tag="vcln1")
            for i, t in enumerate(ts):
                xsrc, off = (xg, t) if t < n_gcn else (xc, t - n_gcn)
                X = wpl.tile([128, 256], F32, tag="X", bufs=2 * GRP + 2)
                _XQ = os.environ.get("XQ", "gpsimd")
                getattr(nc, _XQ).dma_start(X[:], xsrc[off * 128 : (off + 1) * 128, :])
                ln_pre(X[:], vc, i, "ln1")
                states.append({"t": t, "X": X})
            rc, nmr = ln_finish(vc, len(ts), "ln1")
            for i, st in enumerate(states):
                xhat = ln_apply(st["X"][:], 256, vc, rc, i, "ln1", nmr=nmr)
                if XHT_DMA:
                    xhT = dma_transpose(xhat[:], 256, "xhTb")
                    xhT8 = wpl.tile([128, 256], FP8, tag="xhT", bufs=GRP + 2)
                    nc.scalar.copy(xhT8[:], xhT[:])
                else:
                    xhT8 = pe_transpose(xhat[:], 256, "xhT", out_dt=FP8)
                st["xhT"] = xhT8
            return states

        def phase_b(st):
            if st["t"] < n_gcn:
                st["h"] = {}
                for p in ("s", "c"):
                    h_ps = mlp_in(st["xhT"], CS[f"{p}_w18"], CS[f"{p}_b1"][:],
                                  flags[f"{p}_b1"], p)
                    st["h"][p] = gelu_of(h_ps, "h" + p)
            else:
                h_ps = mlp_in(st["xhT"], CS["t_w18"], CS["t_b1"][:],
                              flags["t_b1"], "t")
                st["h"] = gelu_of(h_ps, "hs")

        def phase_c_group(states):
            # C-ln2: all LN2 stats of the group -> one sqrt + one recip
            n_ln2 = sum(2 if st["t"] < n_gcn else 1 for st in states)
            vc2 = sp.tile([128, 2 * n_ln2], F32, tag="vcln2")
            li = 0
            for st in states:
                if st["t"] < n_gcn:
                    for p in ("s", "c"):
                        ln_pre(st["h"][p][:, 512:1024], vc2, li, f"ln2{p}")
                        st[f"li_{p}"] = li
                        li += 1
                else:
                    ln_pre(st["h"][:, 512:1024], vc2, li, "ln2t")
                    st["li_t"] = li
                    li += 1
            rc2, _ = ln_finish(vc2, n_ln2, "ln2f")
            for st in states:
                if st["t"] < n_gcn:
                    st["vhat"] = {
                        p: ln_apply(st["h"][p][:, 512:1024], 512, vc2, rc2,
                                    st[f"li_{p}"], f"ln2{p}")
                        for p in ("s", "c")
                    }
                else:
                    st["vhat"] = ln_apply(st["h"][:, 512:1024], 512, vc2, rc2,
                                          st["li_t"], "ln2t")
            # C-mid: mixers + w2 + residual (no table-sensitive scalar ops)
            for st in states:
                t, X = st["t"], st["X"]
                st["blk"] = {}
                if t < n_gcn:
                    for p in ("s", "c"):
                        u = st["h"][p][:, 0:512]
                        vhat = st["vhat"][p]
                        yt_ps = ppa.tile([128, 512], F32, tag="gwork")
                        for fc in range(4):
                            sl = slice(fc * 128, (fc + 1) * 128)
                            nc.tensor.matmul(
                                yt_ps[:, sl], vhat[:, sl], CS["a_hat"][:],
                                start=True, stop=True,
                            )
                        yt = wp.tile([128, 512], FP8, tag="yt")
                        nc.scalar.copy(yt[:], yt_ps[:])
                        g_ps = ppa.tile([128, 512], F32, tag="gwork")
                        for j in range(2):
                            lhs3 = yt[:, j * 256 : (j + 1) * 256].rearrange(
                                "p (k m) -> p k m", k=2)
                            g3 = CS[f"{p}_grhs8"][:, j * 1024 : (j + 1) * 1024].rearrange(
                                "p (k f) -> p k f", k=2)
                            nc.tensor.matmul(
                                g_ps[:], lhs3, g3,
                                start=(j == 0), stop=False, perf_mode=DR,
                                skip_group_check=(j == 1),
                            )
                        nc.tensor.matmul(
                            g_ps[:], CS[f"{p}_bg_l"][:], CS[f"{p}_bg_r"][:],
                            start=False, stop=True, skip_group_check=True,
                        )
                        st["blk"][p] = backend_mid(g_ps, u, X, p)
                else:
                    u = st["h"][:, 0:512]
                    if flags["t_sg"]:
                        vs = wp.tile([128, 512], BF16, tag="vs")
                        nc.vector.tensor_tensor(vs[:], st["vhat"][:], CS["t_sg"][:], op=ALU.mult)
                    else:
                        vs = st["vhat"]
                    gc_ps = ppa.tile([128, 512], F32, tag="gwork")
                    nc.tensor.matmul(gc_ps[:, 0:512], CS["t_cw1"][:], vs[:, 0:512],
                                     start=True, stop=False)
                    nc.tensor.matmul(gc_ps[:, 1:512], CS["t_cw0"][:], vs[:, 0:511],
                                     start=False, stop=False, skip_group_check=True)
                    nc.tensor.matmul(gc_ps[:, 0:511], CS["t_cw2"][:], vs[:, 1:512],
                                     start=False, stop=False, skip_group_check=True)
                    nc.tensor.matmul(gc_ps[:, 0:512], CS["t_cb_l"][:], CS["t_cb_r"][:],
                                     start=False, stop=True, skip_group_check=True)
                    st["blk"]["t"] = backend_mid(gc_ps, u, X, "t")
            # C-ln3: all LN3 stats -> one sqrt + one recip
            n_ln3 = sum(len(st["blk"]) for st in states)
            vc3 = sp.tile([128, 2 * n_ln3], F32, tag="vcln3")
            li = 0
            for st in states:
                for p, blk in st["blk"].items():
                    ln_pre(blk[:], vc3, li, f"ln3{p}")
                    st[f"l3_{p}"] = li
                    li += 1
            rc3, nmr3 = ln_finish(vc3, n_ln3, "ln3")
            for st in states:
                st["xsh"] = {}
                for p, blk in st["blk"].items():
                    st["xsh"][p] = ln_apply(blk[:], 256, vc3, rc3,
                                            st[f"l3_{p}"], f"ln3{p}", nmr=nmr3)
            # C-fin: final projections + residual + stores
            for st in states:
                t, X = st["t"], st["X"]
                if t < n_gcn:
                    fin_ps = pp.tile([128, 256], F32, tag="finps")
                    for bi, p in enumerate(("s", "c")):
                        xshT = pe_transpose(st["xsh"][p][:], 256, "xshT",
                                            evac=nc.scalar.copy)
                        for cc in range(2):
                            sl = slice(cc * 128, (cc + 1) * 128)
                            nc.tensor.matmul(
                                fin_ps[:], xshT[:, sl], CS[f"{p}_wpr"][cc][:],
                                start=(bi == 0 and cc == 0),
                                stop=(bi == 1 and cc == 1 and not flags["bout"]),
                                skip_group_check=True,
                            )
                    if flags["bout"]:
                        nc.tensor.matmul(
                            fin_ps[:], CS["ones_row"][:], CS["bout"][:],
                            start=False, stop=True, skip_group_check=True,
                        )
                    outt = wp.tile([128, 256], F32, tag="outt")
                    nc.vector.tensor_tensor(outt[:], fin_ps[:], X[:], op=ALU.add)
                    getattr(nc, os.environ.get("OQ", "gpsimd")).dma_start(og[t * 128 : (t + 1) * 128, :], outt[:])
                else:
                    i = t - n_gcn
                    xshT = pe_transpose(st["xsh"]["t"][:], 256, "xshT",
                                        evac=nc.scalar.copy)
                    oc_ps = pps.tile([128, 256], F32, tag="smallps")
                    for cc in range(2):
                        sl = slice(cc * 128, (cc + 1) * 128)
                        nc.tensor.matmul(
                            oc_ps[:], xshT[:, sl], CS["t_wpr"][cc][:],
                            start=(cc == 0), stop=(cc == 1), skip_group_check=True,
                        )
                    occ = wp.tile([128, 256], F32, tag="outt")
                    nc.scalar.copy(occ[:], oc_ps[:])
                    getattr(nc, os.environ.get("OQ", "gpsimd")).dma_start(oc[i * 128 : (i + 1) * 128, :], occ[:])

        n_tiles = n_gcn + n_conv
        if os.environ.get("TILE_ORDER", "seq") == "mix" and n_gcn == n_conv:
            order = [t for i in range(n_gcn) for t in (i, n_gcn + i)]
        else:
            order = list(range(n_tiles))
        pending = phase_a_group([order[t] for t in range(0, min(GRP, n_tiles))])
        for name in _prio:
            if name not in CS:
                _load_const(name)
        for g0 in range(GRP, n_tiles, GRP):
            cur = phase_a_group([order[t] for t in range(g0, min(g0 + GRP, n_tiles))])
            for st in pending:
                phase_b(st)
            phase_c_group(pending)
            pending = cur
        for st in pending:
            phase_b(st)
        phase_c_group(pending)


def build(flags, n_gcn=N_GCN, n_conv=N_CONV):
    nc = bass.Bass()
    _emit(nc, n_gcn, n_conv, flags)
    _split_multi_waits(nc)
    return nc


def kernel(**inputs):
    consts, flags = _host_prep(inputs)
    x = np.ascontiguousarray(np.asarray(inputs["x"], dtype=np.float32))
    xg_full = x.reshape(B * T, N, D)
    xc_full = np.ascontiguousarray(x.transpose(0, 2, 1, 3)).reshape(B * N, T, D)

    nc = build(flags)
    in_maps = []
    for k in range(NCORES):
        m = dict(consts)
        m["xg"] = np.ascontiguousarray(xg_full[32 * k : 32 * (k + 1)]).reshape(N_GCN * 128, 256)
        m["xc"] = np.ascontiguousarray(xc_full[64 * k : 64 * (k + 1)]).reshape(N_CONV * 128, 256)
        in_maps.append(m)
    trace = os.environ.get("BASS_KERNEL_TRACE") == "1"
    res = run_bass_kernel_spmd(nc, in_maps, core_ids=list(range(NCORES)), trace=trace)
    if trace and res.exec_time_ns is not None:
        print(f"HW exec time: {res.exec_time_ns} ns")
    kernel.last_result = res
    og_full = np.stack([r["og"] for r in res.results]).reshape(B * T, N, D).reshape(B, T, N, D)
    oc_full = (
        np.stack([r["oc"] for r in res.results])
        .reshape(B * N, T, D)
        .reshape(B, N, T, D)
        .transpose(0, 2, 1, 3)
    )
    return (og_full + oc_full).astype(np.float32)


# revision 31
# speedup vs baseline: 1.1149x; 1.1149x over previous
"""Trainium2 Bass kernel for nn_Cy2Mixer_layer (gMLP block with conv/GCN/GCN
spatial mixers + fused output projection).

Sharding (8 cores):
  - The two GCN branches (sgu, cgu) + final projection/bias/residual are
    data-parallel over (B*T): 256 token-groups -> 32 per core, each a
    [N=128, D=256] tile (tokens on partitions).
  - The conv branch (tgu, Conv2d(T,T,(1,3)) channel mixer) needs full T per
    (b, n), so it is data-parallel over (B*N): 512 rows -> 64 per core,
    processed as 32 tiles of 2 rows ([2*T=128, D=256], tokens on partitions;
    the T-channel mix is a block-diagonal [128,128] matmul).
  Core outputs: og = xs/xc projections + b_out + residual (bt-sharded) and
  oc = xt projection (bn-sharded); the host scatters and adds the two.

v2: the two large-contraction matmul groups (w1: K=256, gcn-lin: K=512) run
as fp8e4m3 DoubleRow matmuls (2 stacked K-tiles per pass). Weights carry a
power-of-2 scale chosen on the host so 0.02-magnitude weights sit in fp8's
normal range; the scale is removed for free downstream (gelu's input scale
for w1, a pre-divided w2 for the gcn-lin). Bias matmuls whose vectors are
exactly zero for the given inputs are not emitted. Transposes run on the PE
(identity matmul) with GpSimd PSUM-evacuation instead of the DMA XBAR,
freeing the sync queue; LN applies/residual adds are spread across
DVE/GpSimd to balance the elementwise load.
"""

import os
import sys
from contextlib import ExitStack

for _p in ("/opt/trn_rl_repo", "/root/.axon_site/_ro/trn_rl_repo"):
    if os.path.isdir(_p) and _p not in sys.path:
        sys.path.insert(0, _p)

import numpy as np

import bass_rust
import concourse.bass as bass
import concourse.tile as tile
from concourse import mybir
from concourse.bass_utils import run_bass_kernel_spmd

if os.environ.get("LDW_OPT") == "1":
    from concourse import bass_utils as _bu
    _orig_run_command = _bu.run_command

    def _run_command_ldw(cmd, **kw):
        cmd = ["--enable-ldw-opt=true" if c == "--enable-ldw-opt=false" else c
               for c in cmd]
        return _orig_run_command(cmd, **kw)

    _bu.run_command = _run_command_ldw

AF = mybir.ActivationFunctionType
ALU = mybir.AluOpType
F32 = mybir.dt.float32
BF16 = mybir.dt.bfloat16
FP8 = mybir.dt.float8e4
LN_EPS = 1e-5

B, T, N, D, F = 4, 64, 128, 256, 512
NCORES = 8
N_GCN = 32   # bt tiles per core
N_CONV = 32  # conv tiles per core (2 bn rows each)

W1_SCALE = 64.0   # folded out via gelu's input scale
GR_SCALE = 8.0    # folded out via pre-divided w2
FIN_SCALE = 4.0   # split scale: xsh carries 1/4 (via rstd), wpr carries x4

_ctr = [0]


def _split_multi_waits(nc):
    """This walrus build rejects any instruction carrying >1 sync wait
    ("Too many sync wait commands"). Hoist all-but-one wait of every
    instruction onto dedicated same-engine NOPs inserted before it."""
    for f in nc.m.functions:
        for bb in f.blocks:
            insts = bb.instructions
            i = 0
            while i < len(insts):
                inst = insts[i]
                si = inst.sync_info
                if si is not None and si.on_wait is not None and len(si.on_wait) > 1:
                    waits = list(si.on_wait)
                    upd = list(si.on_update) if si.on_update is not None else []
                    inst.sync_info = bass_rust.SyncInfo(
                        on_wait=[waits[-1]], on_update=upd
                    )
                    for w in waits[:-1]:
                        _ctr[0] += 1
                        nop = mybir.InstNoOp(
                            name=f"wsplit-{_ctr[0]}",
                            sync_info=mybir.SyncInfo(on_wait=[w], on_update=[]),
                            bass_nofuse=True,
                            engine=inst.engine,
                        )
                        insts.insert(i, nop)
                        i += 1
                i += 1


def _fp8(x):
    dt = np.dtype(mybir.dt.np(FP8))
    return np.clip(np.asarray(x, np.float32), -240.0, 240.0).astype(dt)


def _dr_pack(w):
    """[K=256 or 512, Fout] -> fp8 DoubleRow layout [128, J, 2, Fout] flattened
    to [128, J*2*Fout] where pair j covers K-chunks (2j, 2j+1)."""
    K, Fo = w.shape
    nj = K // 256
    out = np.zeros((128, nj, 2, Fo), np.float32)
    for j in range(nj):
        for k in range(2):
            out[:, j, k, :] = w[(2 * j + k) * 128 : (2 * j + k + 1) * 128, :]
    return out.reshape(128, nj * 2 * Fo)


def _dr_pack_w1(w1p):
    """[D=256, 2F=1024] -> [128, 2048] with layout [p, (half, k, f512)]:
    half j selects the output 512-block, k the D-chunk of the contraction."""
    return np.concatenate(
        [_dr_pack(w1p[:, 0:512]), _dr_pack(w1p[:, 512:1024])], axis=1
    )


def _host_prep(inp):
    """Fold LN affines into weights; build matmul-ready constant layouts."""
    f32 = np.float32
    bf = np.dtype(mybir.dt.np(BF16))
    c = {}
    flags = {}
    cir = np.asarray(inp["cirmat"])
    a = (cir != 0).astype(f32)
    np.fill_diagonal(a, 1.0)
    deg = a.sum(0).astype(f32)
    dinv = (1.0 / np.sqrt(deg)).astype(f32)
    a_hat = (a * dinv[:, None] * dinv[None, :]).astype(f32)
    c["a_hat"] = a_hat
    colsum = a_hat.sum(0).astype(f32)

    c["ident"] = np.eye(128, dtype=f32)
    c["eps_col"] = np.full((128, 1), LN_EPS, f32)
    c["eps_col_f"] = np.full((128, 1), LN_EPS * FIN_SCALE * FIN_SCALE, f32)
    c["ones_row"] = np.ones((1, 128), f32)

    w_out = np.asarray(inp["w_out"])
    bout = np.asarray(inp["b_out"]).astype(f32).copy()

    fp8_out = {}

    for p, pre, ng_, nb_, off in (
        ("s", "sgu", "n2_g", "n2_b", 256),
        ("c", "cgu", "n3_g", "n3_b", 512),
    ):
        ng = np.asarray(inp[f"{pre}_ng"])
        nb = np.asarray(inp[f"{pre}_nb"])
        w1 = np.asarray(inp[f"{pre}_w1"])
        b1 = np.asarray(inp[f"{pre}_b1"])
        sg = np.asarray(inp[f"{pre}_sg"])
        sb = np.asarray(inp[f"{pre}_sb"])
        gw = np.asarray(inp[f"{pre}_gw"])
        gb = np.asarray(inp[f"{pre}_gb"])
        w2 = np.asarray(inp[f"{pre}_w2"])
        b2 = np.asarray(inp[f"{pre}_b2"])
        w1p = np.ascontiguousarray((w1 * ng[None, :]).T).astype(f32)  # [D, 2F]
        fp8_out[f"{p}_w18"] = _dr_pack_w1(w1p * W1_SCALE)
        b1f = (b1 + w1 @ nb).astype(f32)
        flags[f"{p}_b1"] = bool(np.any(b1f))
        c[f"{p}_b1"] = (b1f * W1_SCALE)[None, :]
        grhs = np.ascontiguousarray((gw * sg[None, :]).T).astype(f32)  # [F, F]
        fp8_out[f"{p}_grhs8"] = _dr_pack(grhs * GR_SCALE)
        c[f"{p}_bg_l"] = np.stack([colsum, np.ones(128, f32)]).astype(f32)
        c[f"{p}_bg_r"] = (np.stack([gw @ sb, gb]) * GR_SCALE).astype(f32)
        c[f"{p}_w2r"] = np.ascontiguousarray(w2.T).astype(f32) / GR_SCALE
        flags[f"{p}_b2"] = bool(np.any(b2))
        c[f"{p}_b2"] = b2[None, :].astype(f32)
        wsl = w_out[:, off : off + 256]
        c[f"{p}_wpr"] = np.ascontiguousarray((wsl * np.asarray(inp[ng_])[None, :]).T).astype(f32)
        bout = bout + wsl @ np.asarray(inp[nb_])

    ng = np.asarray(inp["tgu_ng"])
    nb = np.asarray(inp["tgu_nb"])
    w1 = np.asarray(inp["tgu_w1"])
    b1 = np.asarray(inp["tgu_b1"])
    sg = np.asarray(inp["tgu_sg"])
    sb = np.asarray(inp["tgu_sb"])
    cw = np.asarray(inp["tgu_cw"])[:, :, 0, :]  # [to, ti, dx]
    cb = np.asarray(inp["tgu_cb"])
    w2 = np.asarray(inp["tgu_w2"])
    b2 = np.asarray(inp["tgu_b2"])
    w1p = np.ascontiguousarray((w1 * ng[None, :]).T).astype(f32)
    fp8_out["t_w18"] = _dr_pack_w1(w1p * W1_SCALE)
    b1f = (b1 + w1 @ nb).astype(f32)
    flags["t_b1"] = bool(np.any(b1f))
    c["t_b1"] = (b1f * W1_SCALE)[None, :]
    for dx in range(3):
        blk = np.zeros((128, 128), f32)
        lh = np.ascontiguousarray(cw[:, :, dx].T)  # [ti, to]
        blk[:64, :64] = lh
        blk[64:, 64:] = lh
        c[f"t_cw{dx}"] = blk
    # conv bias as a rank-4 matmul: cb + sum_dx cwsum_dx[to]*sb[fo+dx-1]
    cwsum = cw.sum(1)  # [to, dx]
    lhs = np.zeros((4, 128), f32)
    lhs[0] = np.concatenate([cb, cb])
    for dx in range(3):
        lhs[1 + dx] = np.concatenate([cwsum[:, dx], cwsum[:, dx]])
    rhs = np.zeros((4, 512), f32)
    rhs[0] = 1.0
    rhs[1, 1:] = sb[:511]   # dx=0 reads sb[fo-1]
    rhs[2] = sb             # dx=1 reads sb[fo]
    rhs[3, :511] = sb[1:]   # dx=2 reads sb[fo+1]
    c["t_cb_l"] = lhs
    c["t_cb_r"] = rhs
    c["t_w2r"] = np.ascontiguousarray(w2.T).astype(f32)
    flags["t_b2"] = bool(np.any(b2))
    c["t_b2"] = b2[None, :].astype(f32)
    wsl = w_out[:, 0:256]
    c["t_wpr"] = np.ascontiguousarray((wsl * np.asarray(inp["n1_g"])[None, :]).T).astype(f32)
    bout = bout + wsl @ np.asarray(inp["n1_b"])
    flags["t_sg"] = bool(np.any(sg != 1.0))
    c["t_sg"] = np.broadcast_to(sg, (128, 512)).astype(f32).copy()
    flags["bout"] = bool(np.any(bout))
    c["bout"] = bout[None, :].astype(f32)

    out = {}
    for k, v in c.items():
        if k.startswith("eps_col"):
            out[k] = np.ascontiguousarray(v, dtype=f32)
        else:
            out[k] = np.ascontiguousarray(v).astype(bf)
    for k, v in fp8_out.items():
        out[k] = _fp8(v)
    return out, flags


# Constant tensors DMA'd to SBUF once.
_WSHAPES = {
    "a_hat": ([128, 128], BF16), "ident": ([128, 128], BF16),
    "ones_row": ([1, 128], BF16), "eps_col": ([128, 1], F32),
    "eps_col_f": ([128, 1], F32),
    "t_w18": ([128, 2048], FP8), "t_b1": ([1, 1024], BF16),
    "t_cw0": ([128, 128], BF16), "t_cw1": ([128, 128], BF16),
    "t_cw2": ([128, 128], BF16),
    "t_cb_l": ([4, 128], BF16), "t_cb_r": ([4, 512], BF16),
    "t_w2r": ([512, 256], BF16), "t_b2": ([1, 256], BF16),
    "t_wpr": ([256, 256], BF16),
    "t_sg": ([128, 512], BF16), "bout": ([1, 256], BF16),
}
for _p in ("s", "c"):
    _WSHAPES.update({
        f"{_p}_w18": ([128, 2048], FP8), f"{_p}_b1": ([1, 1024], BF16),
        f"{_p}_grhs8": ([128, 2048], FP8),
        f"{_p}_bg_l": ([2, 128], BF16), f"{_p}_bg_r": ([2, 512], BF16),
        f"{_p}_w2r": ([512, 256], BF16), f"{_p}_b2": ([1, 256], BF16),
        f"{_p}_wpr": ([256, 256], BF16),
    })

DR = mybir.MatmulPerfMode.DoubleRow


def _emit(nc, n_gcn, n_conv, flags):
    xg = nc.dram_tensor("xg", [n_gcn * 128, 256], F32, kind="ExternalInput")
    xc = nc.dram_tensor("xc", [n_conv * 128, 256], F32, kind="ExternalInput")
    og = nc.dram_tensor("og", [n_gcn * 128, 256], F32, kind="ExternalOutput")
    oc = nc.dram_tensor("oc", [n_conv * 128, 256], F32, kind="ExternalOutput")

    wd = {
        k: nc.dram_tensor(k, shp, dt, kind="ExternalInput")
        for k, (shp, dt) in _WSHAPES.items()
    }

    with tile.TileContext(nc) as tc, ExitStack() as ctx:
        cpool = ctx.enter_context(tc.tile_pool(name="consts", bufs=1))
        wp = ctx.enter_context(tc.tile_pool(name="work", bufs=int(os.environ.get("WP_BUFS", "4"))))
        wpl = ctx.enter_context(tc.tile_pool(name="workl", bufs=int(os.environ.get("WPL_BUFS", "10"))))
        sp = ctx.enter_context(tc.tile_pool(name="stats", bufs=int(os.environ.get("SP_BUFS", "8"))))
        pp = ctx.enter_context(tc.tile_pool(name="ps", bufs=int(os.environ.get("FIN_BUFS", "1")), space="PSUM"))
        ppa = ctx.enter_context(tc.tile_pool(name="psa", bufs=int(os.environ.get("GW_BUFS", "2")), space="PSUM"))
        ppt = ctx.enter_context(tc.tile_pool(name="pst", bufs=int(os.environ.get("TP_BUFS", "2")), space="PSUM"))
        pps = ctx.enter_context(tc.tile_pool(name="pss", bufs=int(os.environ.get("PSS_BUFS", "1")), space="PSUM"))
        pph = ctx.enter_context(tc.tile_pool(name="psh", bufs=int(os.environ.get("HP_BUFS", "1")), space="PSUM"))

        GRP = int(os.environ.get("GRP", "8"))
        SC_APPLY = os.environ.get("SC_APPLY", "0") == "1"

        # Const loads: order by first use; round-robin dispatch queues.
        _prio = ["eps_col", "ident", "ones_row", "s_w18", "s_b1", "c_w18",
                 "c_b1", "a_hat", "s_grhs8", "s_bg_l", "s_bg_r", "s_w2r",
                 "s_b2", "c_grhs8", "c_bg_l", "c_bg_r", "c_w2r", "c_b2",
                 "s_wpr", "c_wpr", "bout", "t_w18", "t_b1", "t_sg", "t_cw0",
                 "t_cw1", "t_cw2", "t_cb_l", "t_cb_r", "t_w2r", "t_b2",
                 "t_wpr"]
        _qs = [nc.sync, nc.scalar, nc.gpsimd]
        _qi = [0]

        def _cdma(dst, src):
            _qs[_qi[0] % len(_qs)].dma_start(dst, src)
            _qi[0] += 1

        CS = {}

        def _load_const(name):
            d = wd[name]
            dt = d.dtype
            pdim = d.shape[0]
            if pdim <= 128:
                t = cpool.tile(list(d.shape), dt, tag=name)
                _cdma(t[:], d[:])
                CS[name] = t
            else:
                ts = []
                for i in range(pdim // 128):
                    t = cpool.tile([128, d.shape[1]], dt, tag=f"{name}{i}")
                    _cdma(t[:], d[i * 128 : (i + 1) * 128, :])
                    ts.append(t)
                CS[name] = ts

        _load_const("eps_col")
        _load_const("eps_col_f")
        _load_const("ident")

        def ln_pre(xin, vc, i, tag):
            """bn_stats + bn_aggr into columns [2i, 2i+1] of the group's
            shared (mean, var) tile vc."""
            s6 = sp.tile([128, 6], F32, tag=f"s6{tag}")
            nc.vector.bn_stats(s6[:], xin)
            nc.vector.bn_aggr(vc[:, 2 * i : 2 * i + 2], s6[:])

        def ln_finish(vc, k, tag, fin_scaled=False):
            """One sqrt + one reciprocal over the whole group's stat tile.
            Even columns hold means (sqrt/recip of those are garbage but
            never read); odd columns become rstd. With fin_scaled, computes
            sqrt(S^2 var + S^2 eps) = S*std so rc = rstd/S (S=FIN_SCALE) --
            the 1/S that pairs with the x S baked into the fp8 wpr."""
            sc = sp.tile([128, 2 * k], F32, tag=f"sc{tag}")
            if fin_scaled:
                _chain(nc.scalar.activation(
                    sc[:], vc[:, 0 : 2 * k], AF.Sqrt,
                    bias=CS["eps_col_f"][:], scale=FIN_SCALE * FIN_SCALE))
            else:
                _chain(nc.scalar.activation(sc[:], vc[:, 0 : 2 * k], AF.Sqrt,
                                            bias=CS["eps_col"][:]))
            rc = sp.tile([128, 2 * k], F32, tag=f"rc{tag}")
            nc.vector.reciprocal(rc[:], sc[:])
            if not SC_APPLY:
                return rc, None
            vc3 = vc[:, 0 : 2 * k].rearrange("p (k two) -> p k two", two=2)
            rc3 = rc[:, 0 : 2 * k].rearrange("p (k two) -> p k two", two=2)
            mr = sp.tile([128, k], F32, tag=f"mr{tag}")
            nc.vector.tensor_tensor(mr[:], vc3[:, :, 0:1], rc3[:, :, 1:2],
                                    op=ALU.mult)
            nmr = sp.tile([128, k], F32, tag=f"nmr{tag}")
            nc.vector.tensor_scalar(nmr[:], mr[:], scalar1=-1.0, scalar2=None,
                                    op0=ALU.mult)
            return rc, nmr

        def ln_apply(xin, width, vc, rc, i, tag, eng=None, nmr=None):
            """bf16 (x - mean) * rstd."""
            pool = wpl if tag.startswith(("ln2", "ln3")) else wp
            out = pool.tile([128, width], BF16, tag=f"nrm{tag}", bufs=GRP + 2)
            if nmr is not None:
                # scalar engine: x*rstd + (-mean*rstd)
                nc.scalar.activation(
                    out[:], xin, AF.Identity,
                    bias=nmr[:, i : i + 1],
                    scale=rc[:, 2 * i + 1 : 2 * i + 2],
                )
            else:
                (eng or nc.vector).tensor_scalar(
                    out[:], xin, scalar1=vc[:, 2 * i : 2 * i + 1],
                    scalar2=rc[:, 2 * i + 1 : 2 * i + 2],
                    op0=ALU.subtract, op1=ALU.mult,
                )
            return out

        def pe_transpose(xin, width, tag, out_dt=BF16, evac=None):
            """PE identity transpose + engine evacuation (cast on write).
            GpSimd can't touch PSUM, so evac is scalar (default) or DVE."""
            out = (wpl if tag == "xhT" else wp).tile(
                [128, width], out_dt, tag=tag,
                bufs=(GRP + 2) if tag == "xhT" else None)
            tps = ppt.tile([128, 512], BF16, tag="tpose")
            for cc in range(width // 128):
                sl = slice(cc * 128, (cc + 1) * 128)
                nc.tensor.transpose(tps[:, sl], xin[:, sl], CS["ident"][:])
            (evac or nc.scalar.copy)(out[:], tps[:, 0:width])
            return out

        def dma_transpose(xin, width, tag, q="sync"):
            out = (wpl if tag.startswith("xhT") else wp).tile(
                [128, width], BF16, tag=tag,
                bufs=(GRP + 2) if tag.startswith("xhT") else None)
            o3 = out[:].rearrange("p (c q) -> p c q", q=128)
            getattr(nc, q).dma_start_transpose(o3, xin[:])
            return out

        def mlp_in(xhT8, w18, b1, has_b1, tag):
            """fp8 DoubleRow x̂ @ w1p (x W1_SCALE) [+ b1] into PSUM."""
            h_ps = pph.tile([128, 1024], F32, tag="hps")
            lhs3 = xhT8[:].rearrange("p (k m) -> p k m", k=2)
            for j in range(2):
                hj = h_ps[:, j * 512 : (j + 1) * 512]
                w3 = w18[:, j * 1024 : (j + 1) * 1024].rearrange(
                    "p (k f) -> p k f", k=2)
                nc.tensor.matmul(
                    hj, lhs3, w3, start=True, stop=not has_b1,
                    perf_mode=DR,
                )
                if has_b1:
                    nc.tensor.matmul(
                        hj, CS["ones_row"][:], b1[:, j * 512 : (j + 1) * 512],
                        start=False, stop=True, skip_group_check=True,
                    )
            return h_ps

        def gelu_of(h_ps, tag):
            h = wpl.tile([128, 1024], BF16, tag=tag, bufs=GRP + 2)
            _chain(nc.scalar.activation(h[:], h_ps[:], AF.Gelu, scale=1.0 / W1_SCALE))
            return h

        GT_DMA = os.environ.get("GT_DMA", "1") == "1"

        def backend_mid(gated_src, u, X, p):
            """gated = psum*u, transpose, w2 matmul [+ b2] + residual -> blk."""
            gated = wp.tile([128, 512], BF16, tag="gated")
            nc.vector.tensor_tensor(gated[:], gated_src[:], u, op=ALU.mult)
            if GT_DMA:
                gT = dma_transpose(gated[:], 512, "gT")
            else:
                gT = pe_transpose(gated[:], 512, "gT")
            blk_ps = pps.tile([128, 256], F32, tag="smallps")
            has_b2 = flags[f"{p}_b2"]
            for fc in range(4):
                sl = slice(fc * 128, (fc + 1) * 128)
                nc.tensor.matmul(
                    blk_ps[:], gT[:, sl], CS[f"{p}_w2r"][fc][:],
                    start=(fc == 0), stop=(fc == 3 and not has_b2),
                )
            if has_b2:
                nc.tensor.matmul(
                    blk_ps[:], CS["ones_row"][:], CS[f"{p}_b2"][:],
                    start=False, stop=True, skip_group_check=True,
                )
            blk = wpl.tile([128, 256], BF16, tag=f"blk{p}", bufs=GRP + 1)
            nc.vector.tensor_tensor(blk[:], blk_ps[:], X[:], op=ALU.add)
            return blk

        # ---------------- software-pipelined tile stream ----------------
        # Tiles 0..n_gcn-1 are GCN (xg), the rest conv (xc). Processed in
        # groups of GRP with phase A (load+LN1+transpose) of group g emitted
        # before phases B (mlp+gelu) / C (mixers+backends) of group g-1, so
        # the scalar engine sees [sqrt block][gelu block] per group — 2
        # act-table trips per group instead of 2+ per tile. The _chain dep
        # forces that issue order (the tile scheduler is a greedy ready-heap
        # and would otherwise interleave).
        _last_act = [None]

        def _chain(bi):
            if _last_act[0] is not None:
                tile.add_dep_helper(bi.ins, _last_act[0].ins, reason="act-table order")
            _last_act[0] = bi
            return bi

        XHT_DMA = os.environ.get("XHT_DMA", "0") == "1"

        def phase_a_group(ts):
            states = []
            vc = sp.tile([128, 2 * len(ts)], F32, tag="vcln1")
            for i, t in enumerate(ts):
                xsrc, off = (xg, t) if t < n_gcn else (xc, t - n_gcn)
                X = wpl.tile([128, 256], F32, tag="X", bufs=2 * GRP + 2)
                _XQ = os.environ.get("XQ", "gpsimd")
                getattr(nc, _XQ).dma_start(X[:], xsrc[off * 128 : (off + 1) * 128, :])
                ln_pre(X[:], vc, i, "ln1")
                states.append({"t": t, "X": X})
            rc, nmr = ln_finish(vc, len(ts), "ln1")
            for i, st in enumerate(states):
                xhat = ln_apply(st["X"][:], 256, vc, rc, i, "ln1", nmr=nmr)
                if XHT_DMA:
                    xhT = dma_transpose(xhat[:], 256, "xhTb")
                    xhT8 = wpl.tile([128, 256], FP8, tag="xhT", bufs=GRP + 2)
                    nc.scalar.copy(xhT8[:], xhT[:])
                else:
                    xhT8 = pe_transpose(xhat[:], 256, "xhT", out_dt=FP8)
                st["xhT"] = xhT8
            return states

        def phase_b(st):
            if st["t"] < n_gcn:
                st["h"] = {}
                for p in ("s", "c"):
                    h_ps = mlp_in(st["xhT"], CS[f"{p}_w18"], CS[f"{p}_b1"][:],
                                  flags[f"{p}_b1"], p)
                    st["h"][p] = gelu_of(h_ps, "h" + p)
            else:
                h_ps = mlp_in(st["xhT"], CS["t_w18"], CS["t_b1"][:],
                              flags["t_b1"], "t")
                st["h"] = gelu_of(h_ps, "hs")

        def phase_c_group(states):
            # C-ln2: all LN2 stats of the group -> one sqrt + one recip
            n_ln2 = sum(2 if st["t"] < n_gcn else 1 for st in states)
            vc2 = sp.tile([128, 2 * n_ln2], F32, tag="vcln2")
            li = 0
            for st in states:
                if st["t"] < n_gcn:
                    for p in ("s", "c"):
                        ln_pre(st["h"][p][:, 512:1024], vc2, li, f"ln2{p}")
                        st[f"li_{p}"] = li
                        li += 1
                else:
                    ln_pre(st["h"][:, 512:1024], vc2, li, "ln2t")
                    st["li_t"] = li
                    li += 1
            rc2, _ = ln_finish(vc2, n_ln2, "ln2f")
            for st in states:
                if st["t"] < n_gcn:
                    st["vhat"] = {
                        p: ln_apply(st["h"][p][:, 512:1024], 512, vc2, rc2,
                                    st[f"li_{p}"], f"ln2{p}")
                        for p in ("s", "c")
                    }
                else:
                    st["vhat"] = ln_apply(st["h"][:, 512:1024], 512, vc2, rc2,
                                          st["li_t"], "ln2t")
            # C-mid: mixers + w2 + residual (no table-sensitive scalar ops)
            for st in states:
                t, X = st["t"], st["X"]
                st["blk"] = {}
                if t < n_gcn:
                    for p in ("s", "c"):
                        u = st["h"][p][:, 0:512]
                        vhat = st["vhat"][p]
                        yt_ps = ppa.tile([128, 512], F32, tag="gwork")
                        for fc in range(4):
                            sl = slice(fc * 128, (fc + 1) * 128)
                            nc.tensor.matmul(
                                yt_ps[:, sl], vhat[:, sl], CS["a_hat"][:],
                                start=True, stop=True,
                            )
                        yt = wp.tile([128, 512], FP8, tag="yt")
                        nc.scalar.copy(yt[:], yt_ps[:])
                        g_ps = ppa.tile([128, 512], F32, tag="gwork")
                        for j in range(2):
                            lhs3 = yt[:, j * 256 : (j + 1) * 256].rearrange(
                                "p (k m) -> p k m", k=2)
                            g3 = CS[f"{p}_grhs8"][:, j * 1024 : (j + 1) * 1024].rearrange(
                                "p (k f) -> p k f", k=2)
                            nc.tensor.matmul(
                                g_ps[:], lhs3, g3,
                                start=(j == 0), stop=False, perf_mode=DR,
                                skip_group_check=(j == 1),
                            )
                        nc.tensor.matmul(
                            g_ps[:], CS[f"{p}_bg_l"][:], CS[f"{p}_bg_r"][:],
                            start=False, stop=True, skip_group_check=True,
                        )
                        st["blk"][p] = backend_mid(g_ps, u, X, p)
                else:
                    u = st["h"][:, 0:512]
                    if flags["t_sg"]:
                        vs = wp.tile([128, 512], BF16, tag="vs")
                        nc.vector.tensor_tensor(vs[:], st["vhat"][:], CS["t_sg"][:], op=ALU.mult)
                    else:
                        vs = st["vhat"]
                    gc_ps = ppa.tile([128, 512], F32, tag="gwork")
                    nc.tensor.matmul(gc_ps[:, 0:512], CS["t_cw1"][:], vs[:, 0:512],
                                     start=True, stop=False)
                    nc.tensor.matmul(gc_ps[:, 1:512], CS["t_cw0"][:], vs[:, 0:511],
                                     start=False, stop=False, skip_group_check=True)
                    nc.tensor.matmul(gc_ps[:, 0:511], CS["t_cw2"][:], vs[:, 1:512],
                                     start=False, stop=False, skip_group_check=True)
                    nc.tensor.matmul(gc_ps[:, 0:512], CS["t_cb_l"][:], CS["t_cb_r"][:],
                                     start=False, stop=True, skip_group_check=True)
                    st["blk"]["t"] = backend_mid(gc_ps, u, X, "t")
            # C-ln3: all LN3 stats -> one sqrt + one recip
            n_ln3 = sum(len(st["blk"]) for st in states)
            vc3 = sp.tile([128, 2 * n_ln3], F32, tag="vcln3")
            li = 0
            for st in states:
                for p, blk in st["blk"].items():
                    ln_pre(blk[:], vc3, li, f"ln3{p}")
                    st[f"l3_{p}"] = li
                    li += 1
            rc3, nmr3 = ln_finish(vc3, n_ln3, "ln3")
            for st in states:
                st["xsh"] = {}
                for p, blk in st["blk"].items():
                    st["xsh"][p] = ln_apply(blk[:], 256, vc3, rc3,
                                            st[f"l3_{p}"], f"ln3{p}", nmr=nmr3)
            # C-fin: final projections + residual + stores
            for st in states:
                t, X = st["t"], st["X"]
                if t < n_gcn:
                    fin_ps = pp.tile([128, 256], F32, tag="finps")
                    for bi, p in enumerate(("s", "c")):
                        xshT = pe_transpose(st["xsh"][p][:], 256, "xshT",
                                            evac=nc.scalar.copy)
                        for cc in range(2):
                            sl = slice(cc * 128, (cc + 1) * 128)
                            nc.tensor.matmul(
                                fin_ps[:], xshT[:, sl], CS[f"{p}_wpr"][cc][:],
                                start=(bi == 0 and cc == 0),
                                stop=(bi == 1 and cc == 1 and not flags["bout"]),
                                skip_group_check=True,
                            )
                    if flags["bout"]:
                        nc.tensor.matmul(
                            fin_ps[:], CS["ones_row"][:], CS["bout"][:],
                            start=False, stop=True, skip_group_check=True,
                        )
                    outt = wp.tile([128, 256], F32, tag="outt")
                    nc.vector.tensor_tensor(outt[:], fin_ps[:], X[:], op=ALU.add)
                    getattr(nc, os.environ.get("OQ", "gpsimd")).dma_start(og[t * 128 : (t + 1) * 128, :], outt[:])
                else:
                    i = t - n_gcn
                    xshT = pe_transpose(st["xsh"]["t"][:], 256, "xshT",
                                        evac=nc.scalar.copy)
                    oc_ps = pps.tile([128, 256], F32, tag="smallps")
                    for cc in range(2):
                        sl = slice(cc * 128, (cc + 1) * 128)
                        nc.tensor.matmul(
                            oc_ps[:], xshT[:, sl], CS["t_wpr"][cc][:],
                            start=(cc == 0), stop=(cc == 1), skip_group_check=True,
                        )
                    occ = wp.tile([128, 256], F32, tag="outt")
                    nc.scalar.copy(occ[:], oc_ps[:])
                    getattr(nc, os.environ.get("OQ", "gpsimd")).dma_start(oc[i * 128 : (i + 1) * 128, :], occ[:])

        n_tiles = n_gcn + n_conv
        if os.environ.get("TILE_ORDER", "seq") == "mix" and n_gcn == n_conv:
            order = [t for i in range(n_gcn) for t in (i, n_gcn + i)]
        else:
            order = list(range(n_tiles))
        pending = phase_a_group([order[t] for t in range(0, min(GRP, n_tiles))])
        for name in _prio:
            if name not in CS:
                _load_const(name)
        for g0 in range(GRP, n_tiles, GRP):
            cur = phase_a_group([order[t] for t in range(g0, min(g0 + GRP, n_tiles))])
            for st in pending:
                phase_b(st)
            phase_c_group(pending)
            pending = cur
        for st in pending:
            phase_b(st)
        phase_c_group(pending)


def build(flags, n_gcn=N_GCN, n_conv=N_CONV):
    nc = bass.Bass()
    _emit(nc, n_gcn, n_conv, flags)
    _split_multi_waits(nc)
    return nc


def kernel(**inputs):
    consts, flags = _host_prep(inputs)
    x = np.ascontiguousarray(np.asarray(inputs["x"], dtype=np.float32))
    xg_full = x.reshape(B * T, N, D)
    xc_full = np.ascontiguousarray(x.transpose(0, 2, 1, 3)).reshape(B * N, T, D)

    nc = build(flags)
    in_maps = []
    for k in range(NCORES):
        m = dict(consts)
        m["xg"] = np.ascontiguousarray(xg_full[32 * k : 32 * (k + 1)]).reshape(N_GCN * 128, 256)
        m["xc"] = np.ascontiguousarray(xc_full[64 * k : 64 * (k + 1)]).reshape(N_CONV * 128, 256)
        in_maps.append(m)
    trace = os.environ.get("BASS_KERNEL_TRACE") == "1"
    res = run_bass_kernel_spmd(nc, in_maps, core_ids=list(range(NCORES)), trace=trace)
    if trace and res.exec_time_ns is not None:
        print(f"HW exec time: {res.exec_time_ns} ns")
    kernel.last_result = res
    og_full = np.stack([r["og"] for r in res.results]).reshape(B * T, N, D).reshape(B, T, N, D)
    oc_full = (
        np.stack([r["oc"] for r in res.results])
        .reshape(B * N, T, D)
        .reshape(B, N, T, D)
        .transpose(0, 2, 1, 3)
    )
    return (og_full + oc_full).astype(np.float32)


# revision 32
# speedup vs baseline: 1.1823x; 1.0605x over previous
"""Trainium2 Bass kernel for nn_Cy2Mixer_layer (gMLP block with conv/GCN/GCN
spatial mixers + fused output projection).

Sharding (8 cores):
  - The two GCN branches (sgu, cgu) + final projection/bias/residual are
    data-parallel over (B*T): 256 token-groups -> 32 per core, each a
    [N=128, D=256] tile (tokens on partitions).
  - The conv branch (tgu, Conv2d(T,T,(1,3)) channel mixer) needs full T per
    (b, n), so it is data-parallel over (B*N): 512 rows -> 64 per core,
    processed as 32 tiles of 2 rows ([2*T=128, D=256], tokens on partitions;
    the T-channel mix is a block-diagonal [128,128] matmul).
  Core outputs: og = xs/xc projections + b_out + residual (bt-sharded) and
  oc = xt projection (bn-sharded); the host scatters and adds the two.

v2: the two large-contraction matmul groups (w1: K=256, gcn-lin: K=512) run
as fp8e4m3 DoubleRow matmuls (2 stacked K-tiles per pass). Weights carry a
power-of-2 scale chosen on the host so 0.02-magnitude weights sit in fp8's
normal range; the scale is removed for free downstream (gelu's input scale
for w1, a pre-divided w2 for the gcn-lin). Bias matmuls whose vectors are
exactly zero for the given inputs are not emitted. Transposes run on the PE
(identity matmul) with GpSimd PSUM-evacuation instead of the DMA XBAR,
freeing the sync queue; LN applies/residual adds are spread across
DVE/GpSimd to balance the elementwise load.
"""

import os
import sys
from contextlib import ExitStack

for _p in ("/opt/trn_rl_repo", "/root/.axon_site/_ro/trn_rl_repo"):
    if os.path.isdir(_p) and _p not in sys.path:
        sys.path.insert(0, _p)

import numpy as np

import bass_rust
import concourse.bass as bass
import concourse.tile as tile
from concourse import mybir
from concourse.bass_utils import run_bass_kernel_spmd

if os.environ.get("LDW_OPT") == "1":
    from concourse import bass_utils as _bu
    _orig_run_command = _bu.run_command

    def _run_command_ldw(cmd, **kw):
        cmd = ["--enable-ldw-opt=true" if c == "--enable-ldw-opt=false" else c
               for c in cmd]
        return _orig_run_command(cmd, **kw)

    _bu.run_command = _run_command_ldw

AF = mybir.ActivationFunctionType
ALU = mybir.AluOpType
F32 = mybir.dt.float32
BF16 = mybir.dt.bfloat16
FP8 = mybir.dt.float8e4
LN_EPS = 1e-5

B, T, N, D, F = 4, 64, 128, 256, 512
NCORES = 8
N_GCN = 32   # bt tiles per core
N_CONV = 32  # conv tiles per core (2 bn rows each)

W1_SCALE = 64.0   # folded out via gelu's input scale
GR_SCALE = 8.0    # folded out via pre-divided w2
FIN_SCALE = 4.0   # split scale: xsh carries 1/4 (via rstd), wpr carries x4

_ctr = [0]


def _split_multi_waits(nc):
    """This walrus build rejects any instruction carrying >1 sync wait
    ("Too many sync wait commands"). Hoist all-but-one wait of every
    instruction onto dedicated same-engine NOPs inserted before it."""
    for f in nc.m.functions:
        for bb in f.blocks:
            insts = bb.instructions
            i = 0
            while i < len(insts):
                inst = insts[i]
                si = inst.sync_info
                if si is not None and si.on_wait is not None and len(si.on_wait) > 1:
                    waits = list(si.on_wait)
                    upd = list(si.on_update) if si.on_update is not None else []
                    inst.sync_info = bass_rust.SyncInfo(
                        on_wait=[waits[-1]], on_update=upd
                    )
                    for w in waits[:-1]:
                        _ctr[0] += 1
                        nop = mybir.InstNoOp(
                            name=f"wsplit-{_ctr[0]}",
                            sync_info=mybir.SyncInfo(on_wait=[w], on_update=[]),
                            bass_nofuse=True,
                            engine=inst.engine,
                        )
                        insts.insert(i, nop)
                        i += 1
                i += 1


def _fp8(x):
    dt = np.dtype(mybir.dt.np(FP8))
    return np.clip(np.asarray(x, np.float32), -240.0, 240.0).astype(dt)


def _dr_pack(w):
    """[K=256 or 512, Fout] -> fp8 DoubleRow layout [128, J, 2, Fout] flattened
    to [128, J*2*Fout] where pair j covers K-chunks (2j, 2j+1)."""
    K, Fo = w.shape
    nj = K // 256
    out = np.zeros((128, nj, 2, Fo), np.float32)
    for j in range(nj):
        for k in range(2):
            out[:, j, k, :] = w[(2 * j + k) * 128 : (2 * j + k + 1) * 128, :]
    return out.reshape(128, nj * 2 * Fo)


def _dr_pack_w1(w1p):
    """[D=256, 2F=1024] -> [128, 2048] with layout [p, (half, k, f512)]:
    half j selects the output 512-block, k the D-chunk of the contraction."""
    return np.concatenate(
        [_dr_pack(w1p[:, 0:512]), _dr_pack(w1p[:, 512:1024])], axis=1
    )


def _host_prep(inp):
    """Fold LN affines into weights; build matmul-ready constant layouts."""
    f32 = np.float32
    bf = np.dtype(mybir.dt.np(BF16))
    c = {}
    flags = {}
    cir = np.asarray(inp["cirmat"])
    a = (cir != 0).astype(f32)
    np.fill_diagonal(a, 1.0)
    deg = a.sum(0).astype(f32)
    dinv = (1.0 / np.sqrt(deg)).astype(f32)
    a_hat = (a * dinv[:, None] * dinv[None, :]).astype(f32)
    c["a_hat"] = a_hat
    colsum = a_hat.sum(0).astype(f32)

    c["ident"] = np.eye(128, dtype=f32)
    c["eps_col"] = np.full((128, 1), LN_EPS, f32)
    c["eps_col_f"] = np.full((128, 1), LN_EPS * FIN_SCALE * FIN_SCALE, f32)
    c["ones_row"] = np.ones((1, 128), f32)

    w_out = np.asarray(inp["w_out"])
    bout = np.asarray(inp["b_out"]).astype(f32).copy()

    fp8_out = {}

    for p, pre, ng_, nb_, off in (
        ("s", "sgu", "n2_g", "n2_b", 256),
        ("c", "cgu", "n3_g", "n3_b", 512),
    ):
        ng = np.asarray(inp[f"{pre}_ng"])
        nb = np.asarray(inp[f"{pre}_nb"])
        w1 = np.asarray(inp[f"{pre}_w1"])
        b1 = np.asarray(inp[f"{pre}_b1"])
        sg = np.asarray(inp[f"{pre}_sg"])
        sb = np.asarray(inp[f"{pre}_sb"])
        gw = np.asarray(inp[f"{pre}_gw"])
        gb = np.asarray(inp[f"{pre}_gb"])
        w2 = np.asarray(inp[f"{pre}_w2"])
        b2 = np.asarray(inp[f"{pre}_b2"])
        w1p = np.ascontiguousarray((w1 * ng[None, :]).T).astype(f32)  # [D, 2F]
        fp8_out[f"{p}_w18"] = _dr_pack_w1(w1p * W1_SCALE)
        b1f = (b1 + w1 @ nb).astype(f32)
        flags[f"{p}_b1"] = bool(np.any(b1f))
        c[f"{p}_b1"] = (b1f * W1_SCALE)[None, :]
        grhs = np.ascontiguousarray((gw * sg[None, :]).T).astype(f32)  # [F, F]
        fp8_out[f"{p}_grhs8"] = _dr_pack(grhs * GR_SCALE)
        c[f"{p}_bg_l"] = np.stack([colsum, np.ones(128, f32)]).astype(f32)
        c[f"{p}_bg_r"] = (np.stack([gw @ sb, gb]) * GR_SCALE).astype(f32)
        c[f"{p}_w2r"] = np.ascontiguousarray(w2.T).astype(f32) / GR_SCALE
        flags[f"{p}_b2"] = bool(np.any(b2))
        c[f"{p}_b2"] = b2[None, :].astype(f32)
        wsl = w_out[:, off : off + 256]
        c[f"{p}_wpr"] = np.ascontiguousarray((wsl * np.asarray(inp[ng_])[None, :]).T).astype(f32)
        bout = bout + wsl @ np.asarray(inp[nb_])

    ng = np.asarray(inp["tgu_ng"])
    nb = np.asarray(inp["tgu_nb"])
    w1 = np.asarray(inp["tgu_w1"])
    b1 = np.asarray(inp["tgu_b1"])
    sg = np.asarray(inp["tgu_sg"])
    sb = np.asarray(inp["tgu_sb"])
    cw = np.asarray(inp["tgu_cw"])[:, :, 0, :]  # [to, ti, dx]
    cb = np.asarray(inp["tgu_cb"])
    w2 = np.asarray(inp["tgu_w2"])
    b2 = np.asarray(inp["tgu_b2"])
    w1p = np.ascontiguousarray((w1 * ng[None, :]).T).astype(f32)
    fp8_out["t_w18"] = _dr_pack_w1(w1p * W1_SCALE)
    b1f = (b1 + w1 @ nb).astype(f32)
    flags["t_b1"] = bool(np.any(b1f))
    c["t_b1"] = (b1f * W1_SCALE)[None, :]
    for dx in range(3):
        blk = np.zeros((128, 128), f32)
        lh = np.ascontiguousarray(cw[:, :, dx].T)  # [ti, to]
        blk[:64, :64] = lh
        blk[64:, 64:] = lh
        c[f"t_cw{dx}"] = blk
    # conv bias as a rank-4 matmul: cb + sum_dx cwsum_dx[to]*sb[fo+dx-1]
    cwsum = cw.sum(1)  # [to, dx]
    lhs = np.zeros((4, 128), f32)
    lhs[0] = np.concatenate([cb, cb])
    for dx in range(3):
        lhs[1 + dx] = np.concatenate([cwsum[:, dx], cwsum[:, dx]])
    rhs = np.zeros((4, 512), f32)
    rhs[0] = 1.0
    rhs[1, 1:] = sb[:511]   # dx=0 reads sb[fo-1]
    rhs[2] = sb             # dx=1 reads sb[fo]
    rhs[3, :511] = sb[1:]   # dx=2 reads sb[fo+1]
    c["t_cb_l"] = lhs
    c["t_cb_r"] = rhs
    c["t_w2r"] = np.ascontiguousarray(w2.T).astype(f32)
    flags["t_b2"] = bool(np.any(b2))
    c["t_b2"] = b2[None, :].astype(f32)
    wsl = w_out[:, 0:256]
    c["t_wpr"] = np.ascontiguousarray((wsl * np.asarray(inp["n1_g"])[None, :]).T).astype(f32)
    bout = bout + wsl @ np.asarray(inp["n1_b"])
    flags["t_sg"] = bool(np.any(sg != 1.0))
    c["t_sg"] = np.broadcast_to(sg, (128, 512)).astype(f32).copy()
    flags["bout"] = bool(np.any(bout))
    c["bout"] = bout[None, :].astype(f32)

    out = {}
    for k, v in c.items():
        if k.startswith("eps_col"):
            out[k] = np.ascontiguousarray(v, dtype=f32)
        else:
            out[k] = np.ascontiguousarray(v).astype(bf)
    for k, v in fp8_out.items():
        out[k] = _fp8(v)
    return out, flags


# Constant tensors DMA'd to SBUF once.
_WSHAPES = {
    "a_hat": ([128, 128], BF16), "ident": ([128, 128], BF16),
    "ones_row": ([1, 128], BF16), "eps_col": ([128, 1], F32),
    "eps_col_f": ([128, 1], F32),
    "t_w18": ([128, 2048], FP8), "t_b1": ([1, 1024], BF16),
    "t_cw0": ([128, 128], BF16), "t_cw1": ([128, 128], BF16),
    "t_cw2": ([128, 128], BF16),
    "t_cb_l": ([4, 128], BF16), "t_cb_r": ([4, 512], BF16),
    "t_w2r": ([512, 256], BF16), "t_b2": ([1, 256], BF16),
    "t_wpr": ([256, 256], BF16),
    "t_sg": ([128, 512], BF16), "bout": ([1, 256], BF16),
}
for _p in ("s", "c"):
    _WSHAPES.update({
        f"{_p}_w18": ([128, 2048], FP8), f"{_p}_b1": ([1, 1024], BF16),
        f"{_p}_grhs8": ([128, 2048], FP8),
        f"{_p}_bg_l": ([2, 128], BF16), f"{_p}_bg_r": ([2, 512], BF16),
        f"{_p}_w2r": ([512, 256], BF16), f"{_p}_b2": ([1, 256], BF16),
        f"{_p}_wpr": ([256, 256], BF16),
    })

DR = mybir.MatmulPerfMode.DoubleRow


def _emit(nc, n_gcn, n_conv, flags):
    xg = nc.dram_tensor("xg", [n_gcn * 128, 256], F32, kind="ExternalInput")
    xc = nc.dram_tensor("xc", [n_conv * 128, 256], F32, kind="ExternalInput")
    og = nc.dram_tensor("og", [n_gcn * 128, 256], F32, kind="ExternalOutput")
    oc = nc.dram_tensor("oc", [n_conv * 128, 256], F32, kind="ExternalOutput")

    wd = {
        k: nc.dram_tensor(k, shp, dt, kind="ExternalInput")
        for k, (shp, dt) in _WSHAPES.items()
    }

    with tile.TileContext(nc) as tc, ExitStack() as ctx:
        cpool = ctx.enter_context(tc.tile_pool(name="consts", bufs=1))
        wp = ctx.enter_context(tc.tile_pool(name="work", bufs=int(os.environ.get("WP_BUFS", "4"))))
        wpl = ctx.enter_context(tc.tile_pool(name="workl", bufs=int(os.environ.get("WPL_BUFS", "10"))))
        sp = ctx.enter_context(tc.tile_pool(name="stats", bufs=int(os.environ.get("SP_BUFS", "8"))))
        pp = ctx.enter_context(tc.tile_pool(name="ps", bufs=int(os.environ.get("FIN_BUFS", "1")), space="PSUM"))
        ppa = ctx.enter_context(tc.tile_pool(name="psa", bufs=int(os.environ.get("GW_BUFS", "2")), space="PSUM"))
        ppt = ctx.enter_context(tc.tile_pool(name="pst", bufs=int(os.environ.get("TP_BUFS", "2")), space="PSUM"))
        pps = ctx.enter_context(tc.tile_pool(name="pss", bufs=int(os.environ.get("PSS_BUFS", "1")), space="PSUM"))
        pph = ctx.enter_context(tc.tile_pool(name="psh", bufs=int(os.environ.get("HP_BUFS", "1")), space="PSUM"))

        GRP = int(os.environ.get("GRP", "8"))
        SC_APPLY = os.environ.get("SC_APPLY", "0") == "1"

        # Const loads: order by first use; round-robin dispatch queues.
        _prio = ["eps_col", "ident", "ones_row", "s_w18", "s_b1", "c_w18",
                 "c_b1", "a_hat", "s_grhs8", "s_bg_l", "s_bg_r", "s_w2r",
                 "s_b2", "c_grhs8", "c_bg_l", "c_bg_r", "c_w2r", "c_b2",
                 "s_wpr", "c_wpr", "bout", "t_w18", "t_b1", "t_sg", "t_cw0",
                 "t_cw1", "t_cw2", "t_cb_l", "t_cb_r", "t_w2r", "t_b2",
                 "t_wpr"]
        _qs = [nc.sync, nc.scalar, nc.gpsimd]
        _qi = [0]

        def _cdma(dst, src):
            _qs[_qi[0] % len(_qs)].dma_start(dst, src)
            _qi[0] += 1

        CS = {}

        def _load_const(name):
            d = wd[name]
            dt = d.dtype
            pdim = d.shape[0]
            if pdim <= 128:
                t = cpool.tile(list(d.shape), dt, tag=name)
                _cdma(t[:], d[:])
                CS[name] = t
            else:
                ts = []
                for i in range(pdim // 128):
                    t = cpool.tile([128, d.shape[1]], dt, tag=f"{name}{i}")
                    _cdma(t[:], d[i * 128 : (i + 1) * 128, :])
                    ts.append(t)
                CS[name] = ts

        _load_const("eps_col")
        _load_const("eps_col_f")
        _load_const("ident")

        def ln_pre(xin, vc, i, tag):
            """bn_stats + bn_aggr into columns [2i, 2i+1] of the group's
            shared (mean, var) tile vc."""
            s6 = sp.tile([128, 6], F32, tag=f"s6{tag}")
            nc.vector.bn_stats(s6[:], xin)
            nc.vector.bn_aggr(vc[:, 2 * i : 2 * i + 2], s6[:])

        def ln_finish(vc, k, tag, fin_scaled=False):
            """One sqrt + one reciprocal over the whole group's stat tile.
            Even columns hold means (sqrt/recip of those are garbage but
            never read); odd columns become rstd. With fin_scaled, computes
            sqrt(S^2 var + S^2 eps) = S*std so rc = rstd/S (S=FIN_SCALE) --
            the 1/S that pairs with the x S baked into the fp8 wpr."""
            sc = sp.tile([128, 2 * k], F32, tag=f"sc{tag}")
            if fin_scaled:
                _chain(nc.scalar.activation(
                    sc[:], vc[:, 0 : 2 * k], AF.Sqrt,
                    bias=CS["eps_col_f"][:], scale=FIN_SCALE * FIN_SCALE))
            else:
                _chain(nc.scalar.activation(sc[:], vc[:, 0 : 2 * k], AF.Sqrt,
                                            bias=CS["eps_col"][:]))
            rc = sp.tile([128, 2 * k], F32, tag=f"rc{tag}")
            nc.vector.reciprocal(rc[:], sc[:])
            if not SC_APPLY:
                return rc, None
            vc3 = vc[:, 0 : 2 * k].rearrange("p (k two) -> p k two", two=2)
            rc3 = rc[:, 0 : 2 * k].rearrange("p (k two) -> p k two", two=2)
            mr = sp.tile([128, k], F32, tag=f"mr{tag}")
            nc.vector.tensor_tensor(mr[:], vc3[:, :, 0:1], rc3[:, :, 1:2],
                                    op=ALU.mult)
            nmr = sp.tile([128, k], F32, tag=f"nmr{tag}")
            nc.vector.tensor_scalar(nmr[:], mr[:], scalar1=-1.0, scalar2=None,
                                    op0=ALU.mult)
            return rc, nmr

        def ln_apply(xin, width, vc, rc, i, tag, eng=None, nmr=None):
            """bf16 (x - mean) * rstd."""
            pool = wpl if tag.startswith(("ln2", "ln3")) else wp
            out = pool.tile([128, width], BF16, tag=f"nrm{tag}", bufs=GRP + 2)
            if nmr is not None:
                # scalar engine: x*rstd + (-mean*rstd)
                nc.scalar.activation(
                    out[:], xin, AF.Identity,
                    bias=nmr[:, i : i + 1],
                    scale=rc[:, 2 * i + 1 : 2 * i + 2],
                )
            else:
                (eng or nc.vector).tensor_scalar(
                    out[:], xin, scalar1=vc[:, 2 * i : 2 * i + 1],
                    scalar2=rc[:, 2 * i + 1 : 2 * i + 2],
                    op0=ALU.subtract, op1=ALU.mult,
                )
            return out

        def pe_transpose(xin, width, tag, out_dt=BF16, evac=None):
            """PE identity transpose + engine evacuation (cast on write).
            GpSimd can't touch PSUM, so evac is scalar (default) or DVE."""
            out = (wpl if tag == "xhT" else wp).tile(
                [128, width], out_dt, tag=tag,
                bufs=(GRP + 2) if tag == "xhT" else None)
            tps = ppt.tile([128, 512], BF16, tag="tpose")
            for cc in range(width // 128):
                sl = slice(cc * 128, (cc + 1) * 128)
                nc.tensor.transpose(tps[:, sl], xin[:, sl], CS["ident"][:])
            (evac or nc.scalar.copy)(out[:], tps[:, 0:width])
            return out

        def dma_transpose(xin, width, tag, q="sync"):
            out = (wpl if tag.startswith("xhT") else wp).tile(
                [128, width], BF16, tag=tag,
                bufs=(GRP + 2) if tag.startswith("xhT") else None)
            o3 = out[:].rearrange("p (c q) -> p c q", q=128)
            getattr(nc, q).dma_start_transpose(o3, xin[:])
            return out

        def mlp_uv(xhT8, w18, b1, has_b1, tag):
            """fp8 DoubleRow x̂ @ w1p (x W1_SCALE) [+ b1], half at a time
            into separate PSUM banks, gelu'd per half (v first: LN2 needs
            it immediately; u only gates later). -> (u, v) bf16 SBUF."""
            lhs3 = xhT8[:].rearrange("p (k m) -> p k m", k=2)
            out = {}
            for j in (1, 0):  # v-half first
                h_ps = pph.tile([128, 512], F32, tag=("vps" if j else "ups"))
                w3 = w18[:, j * 1024 : (j + 1) * 1024].rearrange(
                    "p (k f) -> p k f", k=2)
                nc.tensor.matmul(
                    h_ps[:], lhs3, w3, start=True, stop=not has_b1,
                    perf_mode=DR,
                )
                if has_b1:
                    nc.tensor.matmul(
                        h_ps[:], CS["ones_row"][:], b1[:, j * 512 : (j + 1) * 512],
                        start=False, stop=True, skip_group_check=True,
                    )
                hv = wpl.tile([128, 512], BF16, tag=("v" if j else "u") + tag,
                              bufs=GRP + 2)
                _chain(nc.scalar.activation(hv[:], h_ps[:], AF.Gelu,
                                            scale=1.0 / W1_SCALE))
                out["v" if j else "u"] = hv
            return out["u"], out["v"]

        GT_DMA = os.environ.get("GT_DMA", "1") == "1"

        def backend_mid(gated_src, u, X, p):
            """gated = psum*u, transpose, w2 matmul [+ b2] + residual -> blk."""
            gated = wp.tile([128, 512], BF16, tag="gated")
            nc.vector.tensor_tensor(gated[:], gated_src[:], u, op=ALU.mult)
            if GT_DMA:
                gT = dma_transpose(gated[:], 512, "gT")
            else:
                gT = pe_transpose(gated[:], 512, "gT")
            blk_ps = pps.tile([128, 256], F32, tag="smallps")
            has_b2 = flags[f"{p}_b2"]
            for fc in range(4):
                sl = slice(fc * 128, (fc + 1) * 128)
                nc.tensor.matmul(
                    blk_ps[:], gT[:, sl], CS[f"{p}_w2r"][fc][:],
                    start=(fc == 0), stop=(fc == 3 and not has_b2),
                )
            if has_b2:
                nc.tensor.matmul(
                    blk_ps[:], CS["ones_row"][:], CS[f"{p}_b2"][:],
                    start=False, stop=True, skip_group_check=True,
                )
            blk = wpl.tile([128, 256], BF16, tag=f"blk{p}", bufs=GRP + 1)
            nc.vector.tensor_tensor(blk[:], blk_ps[:], X[:], op=ALU.add)
            return blk

        # ---------------- software-pipelined tile stream ----------------
        # Tiles 0..n_gcn-1 are GCN (xg), the rest conv (xc). Processed in
        # groups of GRP with phase A (load+LN1+transpose) of group g emitted
        # before phases B (mlp+gelu) / C (mixers+backends) of group g-1, so
        # the scalar engine sees [sqrt block][gelu block] per group — 2
        # act-table trips per group instead of 2+ per tile. The _chain dep
        # forces that issue order (the tile scheduler is a greedy ready-heap
        # and would otherwise interleave).
        _last_act = [None]

        def _chain(bi):
            if _last_act[0] is not None:
                tile.add_dep_helper(bi.ins, _last_act[0].ins, reason="act-table order")
            _last_act[0] = bi
            return bi

        XHT_DMA = os.environ.get("XHT_DMA", "0") == "1"

        def phase_a_group(ts):
            states = []
            vc = sp.tile([128, 2 * len(ts)], F32, tag="vcln1")
            for i, t in enumerate(ts):
                xsrc, off = (xg, t) if t < n_gcn else (xc, t - n_gcn)
                X = wpl.tile([128, 256], F32, tag="X", bufs=2 * GRP + 2)
                _XQ = os.environ.get("XQ", "gpsimd")
                getattr(nc, _XQ).dma_start(X[:], xsrc[off * 128 : (off + 1) * 128, :])
                ln_pre(X[:], vc, i, "ln1")
                states.append({"t": t, "X": X})
            rc, nmr = ln_finish(vc, len(ts), "ln1")
            for i, st in enumerate(states):
                xhat = ln_apply(st["X"][:], 256, vc, rc, i, "ln1", nmr=nmr)
                if XHT_DMA:
                    xhT = dma_transpose(xhat[:], 256, "xhTb")
                    xhT8 = wpl.tile([128, 256], FP8, tag="xhT", bufs=GRP + 2)
                    nc.scalar.copy(xhT8[:], xhT[:])
                else:
                    xhT8 = pe_transpose(xhat[:], 256, "xhT", out_dt=FP8)
                st["xhT"] = xhT8
            return states

        def phase_b(st):
            if st["t"] < n_gcn:
                st["u"], st["v"] = {}, {}
                for p in ("s", "c"):
                    st["u"][p], st["v"][p] = mlp_uv(
                        st["xhT"], CS[f"{p}_w18"], CS[f"{p}_b1"][:],
                        flags[f"{p}_b1"], p)
            else:
                st["u"], st["v"] = mlp_uv(st["xhT"], CS["t_w18"],
                                          CS["t_b1"][:], flags["t_b1"], "t")

        def phase_c_group(states):
            # C-ln2: all LN2 stats of the group -> one sqrt + one recip
            n_ln2 = sum(2 if st["t"] < n_gcn else 1 for st in states)
            vc2 = sp.tile([128, 2 * n_ln2], F32, tag="vcln2")
            li = 0
            for st in states:
                if st["t"] < n_gcn:
                    for p in ("s", "c"):
                        ln_pre(st["v"][p][:], vc2, li, f"ln2{p}")
                        st[f"li_{p}"] = li
                        li += 1
                else:
                    ln_pre(st["v"][:], vc2, li, "ln2t")
                    st["li_t"] = li
                    li += 1
            rc2, _ = ln_finish(vc2, n_ln2, "ln2f")
            for st in states:
                if st["t"] < n_gcn:
                    st["vhat"] = {
                        p: ln_apply(st["v"][p][:], 512, vc2, rc2,
                                    st[f"li_{p}"], f"ln2{p}")
                        for p in ("s", "c")
                    }
                else:
                    st["vhat"] = ln_apply(st["v"][:], 512, vc2, rc2,
                                          st["li_t"], "ln2t")
            # C-mid: mixers + w2 + residual (no table-sensitive scalar ops)
            for st in states:
                t, X = st["t"], st["X"]
                st["blk"] = {}
                if t < n_gcn:
                    for p in ("s", "c"):
                        u = st["u"][p][:]
                        vhat = st["vhat"][p]
                        yt_ps = ppa.tile([128, 512], F32, tag="gwork")
                        for fc in range(4):
                            sl = slice(fc * 128, (fc + 1) * 128)
                            nc.tensor.matmul(
                                yt_ps[:, sl], vhat[:, sl], CS["a_hat"][:],
                                start=True, stop=True,
                            )
                        yt = wp.tile([128, 512], FP8, tag="yt")
                        nc.scalar.copy(yt[:], yt_ps[:])
                        g_ps = ppa.tile([128, 512], F32, tag="gwork")
                        for j in range(2):
                            lhs3 = yt[:, j * 256 : (j + 1) * 256].rearrange(
                                "p (k m) -> p k m", k=2)
                            g3 = CS[f"{p}_grhs8"][:, j * 1024 : (j + 1) * 1024].rearrange(
                                "p (k f) -> p k f", k=2)
                            nc.tensor.matmul(
                                g_ps[:], lhs3, g3,
                                start=(j == 0), stop=False, perf_mode=DR,
                                skip_group_check=(j == 1),
                            )
                        nc.tensor.matmul(
                            g_ps[:], CS[f"{p}_bg_l"][:], CS[f"{p}_bg_r"][:],
                            start=False, stop=True, skip_group_check=True,
                        )
                        st["blk"][p] = backend_mid(g_ps, u, X, p)
                else:
                    u = st["u"][:]
                    if flags["t_sg"]:
                        vs = wp.tile([128, 512], BF16, tag="vs")
                        nc.vector.tensor_tensor(vs[:], st["vhat"][:], CS["t_sg"][:], op=ALU.mult)
                    else:
                        vs = st["vhat"]
                    gc_ps = ppa.tile([128, 512], F32, tag="gwork")
                    nc.tensor.matmul(gc_ps[:, 0:512], CS["t_cw1"][:], vs[:, 0:512],
                                     start=True, stop=False)
                    nc.tensor.matmul(gc_ps[:, 1:512], CS["t_cw0"][:], vs[:, 0:511],
                                     start=False, stop=False, skip_group_check=True)
                    nc.tensor.matmul(gc_ps[:, 0:511], CS["t_cw2"][:], vs[:, 1:512],
                                     start=False, stop=False, skip_group_check=True)
                    nc.tensor.matmul(gc_ps[:, 0:512], CS["t_cb_l"][:], CS["t_cb_r"][:],
                                     start=False, stop=True, skip_group_check=True)
                    st["blk"]["t"] = backend_mid(gc_ps, u, X, "t")
            # C-ln3: all LN3 stats -> one sqrt + one recip
            n_ln3 = sum(len(st["blk"]) for st in states)
            vc3 = sp.tile([128, 2 * n_ln3], F32, tag="vcln3")
            li = 0
            for st in states:
                for p, blk in st["blk"].items():
                    ln_pre(blk[:], vc3, li, f"ln3{p}")
                    st[f"l3_{p}"] = li
                    li += 1
            rc3, nmr3 = ln_finish(vc3, n_ln3, "ln3")
            for st in states:
                st["xsh"] = {}
                for p, blk in st["blk"].items():
                    st["xsh"][p] = ln_apply(blk[:], 256, vc3, rc3,
                                            st[f"l3_{p}"], f"ln3{p}", nmr=nmr3)
            # C-fin: final projections + residual + stores
            for st in states:
                t, X = st["t"], st["X"]
                if t < n_gcn:
                    fin_ps = pp.tile([128, 256], F32, tag="finps")
                    for bi, p in enumerate(("s", "c")):
                        xshT = pe_transpose(st["xsh"][p][:], 256, "xshT",
                                            evac=nc.scalar.copy)
                        for cc in range(2):
                            sl = slice(cc * 128, (cc + 1) * 128)
                            nc.tensor.matmul(
                                fin_ps[:], xshT[:, sl], CS[f"{p}_wpr"][cc][:],
                                start=(bi == 0 and cc == 0),
                                stop=(bi == 1 and cc == 1 and not flags["bout"]),
                                skip_group_check=True,
                            )
                    if flags["bout"]:
                        nc.tensor.matmul(
                            fin_ps[:], CS["ones_row"][:], CS["bout"][:],
                            start=False, stop=True, skip_group_check=True,
                        )
                    outt = wp.tile([128, 256], F32, tag="outt")
                    nc.vector.tensor_tensor(outt[:], fin_ps[:], X[:], op=ALU.add)
                    getattr(nc, os.environ.get("OQ", "gpsimd")).dma_start(og[t * 128 : (t + 1) * 128, :], outt[:])
                else:
                    i = t - n_gcn
                    xshT = pe_transpose(st["xsh"]["t"][:], 256, "xshT",
                                        evac=nc.scalar.copy)
                    oc_ps = pps.tile([128, 256], F32, tag="smallps")
                    for cc in range(2):
                        sl = slice(cc * 128, (cc + 1) * 128)
                        nc.tensor.matmul(
                            oc_ps[:], xshT[:, sl], CS["t_wpr"][cc][:],
                            start=(cc == 0), stop=(cc == 1), skip_group_check=True,
                        )
                    occ = wp.tile([128, 256], F32, tag="outt")
                    nc.scalar.copy(occ[:], oc_ps[:])
                    getattr(nc, os.environ.get("OQ", "gpsimd")).dma_start(oc[i * 128 : (i + 1) * 128, :], occ[:])

        n_tiles = n_gcn + n_conv
        if os.environ.get("TILE_ORDER", "seq") == "mix" and n_gcn == n_conv:
            order = [t for i in range(n_gcn) for t in (i, n_gcn + i)]
        else:
            order = list(range(n_tiles))
        pending = phase_a_group([order[t] for t in range(0, min(GRP, n_tiles))])
        for name in _prio:
            if name not in CS:
                _load_const(name)
        for g0 in range(GRP, n_tiles, GRP):
            cur = phase_a_group([order[t] for t in range(g0, min(g0 + GRP, n_tiles))])
            for st in pending:
                phase_b(st)
            phase_c_group(pending)
            pending = cur
        for st in pending:
            phase_b(st)
        phase_c_group(pending)


def build(flags, n_gcn=N_GCN, n_conv=N_CONV):
    nc = bass.Bass()
    _emit(nc, n_gcn, n_conv, flags)
    _split_multi_waits(nc)
    return nc


def kernel(**inputs):
    consts, flags = _host_prep(inputs)
    x = np.ascontiguousarray(np.asarray(inputs["x"], dtype=np.float32))
    xg_full = x.reshape(B * T, N, D)
    xc_full = np.ascontiguousarray(x.transpose(0, 2, 1, 3)).reshape(B * N, T, D)

    nc = build(flags)
    in_maps = []
    for k in range(NCORES):
        m = dict(consts)
        m["xg"] = np.ascontiguousarray(xg_full[32 * k : 32 * (k + 1)]).reshape(N_GCN * 128, 256)
        m["xc"] = np.ascontiguousarray(xc_full[64 * k : 64 * (k + 1)]).reshape(N_CONV * 128, 256)
        in_maps.append(m)
    trace = os.environ.get("BASS_KERNEL_TRACE") == "1"
    res = run_bass_kernel_spmd(nc, in_maps, core_ids=list(range(NCORES)), trace=trace)
    if trace and res.exec_time_ns is not None:
        print(f"HW exec time: {res.exec_time_ns} ns")
    kernel.last_result = res
    og_full = np.stack([r["og"] for r in res.results]).reshape(B * T, N, D).reshape(B, T, N, D)
    oc_full = (
        np.stack([r["oc"] for r in res.results])
        .reshape(B * N, T, D)
        .reshape(B, N, T, D)
        .transpose(0, 2, 1, 3)
    )
    return (og_full + oc_full).astype(np.float32)


# revision 33
# speedup vs baseline: 1.2101x; 1.0235x over previous
"""Trainium2 Bass kernel for nn_Cy2Mixer_layer (gMLP block with conv/GCN/GCN
spatial mixers + fused output projection).

Sharding (8 cores):
  - The two GCN branches (sgu, cgu) + final projection/bias/residual are
    data-parallel over (B*T): 256 token-groups -> 32 per core, each a
    [N=128, D=256] tile (tokens on partitions).
  - The conv branch (tgu, Conv2d(T,T,(1,3)) channel mixer) needs full T per
    (b, n), so it is data-parallel over (B*N): 512 rows -> 64 per core,
    processed as 32 tiles of 2 rows ([2*T=128, D=256], tokens on partitions;
    the T-channel mix is a block-diagonal [128,128] matmul).
  Core outputs: og = xs/xc projections + b_out + residual (bt-sharded) and
  oc = xt projection (bn-sharded); the host scatters and adds the two.

v2: the two large-contraction matmul groups (w1: K=256, gcn-lin: K=512) run
as fp8e4m3 DoubleRow matmuls (2 stacked K-tiles per pass). Weights carry a
power-of-2 scale chosen on the host so 0.02-magnitude weights sit in fp8's
normal range; the scale is removed for free downstream (gelu's input scale
for w1, a pre-divided w2 for the gcn-lin). Bias matmuls whose vectors are
exactly zero for the given inputs are not emitted. Transposes run on the PE
(identity matmul) with GpSimd PSUM-evacuation instead of the DMA XBAR,
freeing the sync queue; LN applies/residual adds are spread across
DVE/GpSimd to balance the elementwise load.
"""

import os
import sys
from contextlib import ExitStack

for _p in ("/opt/trn_rl_repo", "/root/.axon_site/_ro/trn_rl_repo"):
    if os.path.isdir(_p) and _p not in sys.path:
        sys.path.insert(0, _p)

import numpy as np

import bass_rust
import concourse.bass as bass
import concourse.tile as tile
from concourse import mybir
from concourse.bass_utils import run_bass_kernel_spmd

if os.environ.get("LDW_OPT") == "1":
    from concourse import bass_utils as _bu
    _orig_run_command = _bu.run_command

    def _run_command_ldw(cmd, **kw):
        cmd = ["--enable-ldw-opt=true" if c == "--enable-ldw-opt=false" else c
               for c in cmd]
        return _orig_run_command(cmd, **kw)

    _bu.run_command = _run_command_ldw

AF = mybir.ActivationFunctionType
ALU = mybir.AluOpType
F32 = mybir.dt.float32
BF16 = mybir.dt.bfloat16
FP8 = mybir.dt.float8e4
LN_EPS = 1e-5

B, T, N, D, F = 4, 64, 128, 256, 512
NCORES = 8
N_GCN = 32   # bt tiles per core
N_CONV = 32  # conv tiles per core (2 bn rows each)

W1_SCALE = 64.0   # folded out via gelu's input scale
GR_SCALE = 8.0    # folded out via pre-divided w2
FIN_SCALE = 4.0   # split scale: xsh carries 1/4 (via rstd), wpr carries x4

_ctr = [0]


def _split_multi_waits(nc):
    """This walrus build rejects any instruction carrying >1 sync wait
    ("Too many sync wait commands"). Hoist all-but-one wait of every
    instruction onto dedicated same-engine NOPs inserted before it."""
    for f in nc.m.functions:
        for bb in f.blocks:
            insts = bb.instructions
            i = 0
            while i < len(insts):
                inst = insts[i]
                si = inst.sync_info
                if si is not None and si.on_wait is not None and len(si.on_wait) > 1:
                    waits = list(si.on_wait)
                    upd = list(si.on_update) if si.on_update is not None else []
                    inst.sync_info = bass_rust.SyncInfo(
                        on_wait=[waits[-1]], on_update=upd
                    )
                    for w in waits[:-1]:
                        _ctr[0] += 1
                        nop = mybir.InstNoOp(
                            name=f"wsplit-{_ctr[0]}",
                            sync_info=mybir.SyncInfo(on_wait=[w], on_update=[]),
                            bass_nofuse=True,
                            engine=inst.engine,
                        )
                        insts.insert(i, nop)
                        i += 1
                i += 1


def _fp8(x):
    dt = np.dtype(mybir.dt.np(FP8))
    return np.clip(np.asarray(x, np.float32), -240.0, 240.0).astype(dt)


def _dr_pack(w):
    """[K=256 or 512, Fout] -> fp8 DoubleRow layout [128, J, 2, Fout] flattened
    to [128, J*2*Fout] where pair j covers K-chunks (2j, 2j+1)."""
    K, Fo = w.shape
    nj = K // 256
    out = np.zeros((128, nj, 2, Fo), np.float32)
    for j in range(nj):
        for k in range(2):
            out[:, j, k, :] = w[(2 * j + k) * 128 : (2 * j + k + 1) * 128, :]
    return out.reshape(128, nj * 2 * Fo)


def _dr_pack_w1(w1p):
    """[D=256, 2F=1024] -> [128, 2048] with layout [p, (half, k, f512)]:
    half j selects the output 512-block, k the D-chunk of the contraction."""
    return np.concatenate(
        [_dr_pack(w1p[:, 0:512]), _dr_pack(w1p[:, 512:1024])], axis=1
    )


def _host_prep(inp):
    """Fold LN affines into weights; build matmul-ready constant layouts."""
    f32 = np.float32
    bf = np.dtype(mybir.dt.np(BF16))
    c = {}
    flags = {}
    cir = np.asarray(inp["cirmat"])
    a = (cir != 0).astype(f32)
    np.fill_diagonal(a, 1.0)
    deg = a.sum(0).astype(f32)
    dinv = (1.0 / np.sqrt(deg)).astype(f32)
    a_hat = (a * dinv[:, None] * dinv[None, :]).astype(f32)
    c["a_hat"] = a_hat
    colsum = a_hat.sum(0).astype(f32)

    c["ident"] = np.eye(128, dtype=f32)
    c["eps_col"] = np.full((128, 1), LN_EPS, f32)
    c["eps_col_f"] = np.full((128, 1), LN_EPS * FIN_SCALE * FIN_SCALE, f32)
    c["ones_row"] = np.ones((1, 128), f32)

    w_out = np.asarray(inp["w_out"])
    bout = np.asarray(inp["b_out"]).astype(f32).copy()

    fp8_out = {}

    for p, pre, ng_, nb_, off in (
        ("s", "sgu", "n2_g", "n2_b", 256),
        ("c", "cgu", "n3_g", "n3_b", 512),
    ):
        ng = np.asarray(inp[f"{pre}_ng"])
        nb = np.asarray(inp[f"{pre}_nb"])
        w1 = np.asarray(inp[f"{pre}_w1"])
        b1 = np.asarray(inp[f"{pre}_b1"])
        sg = np.asarray(inp[f"{pre}_sg"])
        sb = np.asarray(inp[f"{pre}_sb"])
        gw = np.asarray(inp[f"{pre}_gw"])
        gb = np.asarray(inp[f"{pre}_gb"])
        w2 = np.asarray(inp[f"{pre}_w2"])
        b2 = np.asarray(inp[f"{pre}_b2"])
        w1p = np.ascontiguousarray((w1 * ng[None, :]).T).astype(f32)  # [D, 2F]
        fp8_out[f"{p}_w18"] = _dr_pack_w1(w1p * W1_SCALE)
        b1f = (b1 + w1 @ nb).astype(f32)
        flags[f"{p}_b1"] = bool(np.any(b1f))
        c[f"{p}_b1"] = (b1f * W1_SCALE)[None, :]
        grhs = np.ascontiguousarray((gw * sg[None, :]).T).astype(f32)  # [F, F]
        fp8_out[f"{p}_grhs8"] = _dr_pack(grhs * GR_SCALE)
        c[f"{p}_bg_l"] = np.stack([colsum, np.ones(128, f32)]).astype(f32)
        c[f"{p}_bg_r"] = (np.stack([gw @ sb, gb]) * GR_SCALE).astype(f32)
        c[f"{p}_w2r"] = np.ascontiguousarray(w2.T).astype(f32) / GR_SCALE
        flags[f"{p}_b2"] = bool(np.any(b2))
        c[f"{p}_b2"] = b2[None, :].astype(f32)
        wsl = w_out[:, off : off + 256]
        c[f"{p}_wpr"] = np.ascontiguousarray((wsl * np.asarray(inp[ng_])[None, :]).T).astype(f32)
        bout = bout + wsl @ np.asarray(inp[nb_])

    ng = np.asarray(inp["tgu_ng"])
    nb = np.asarray(inp["tgu_nb"])
    w1 = np.asarray(inp["tgu_w1"])
    b1 = np.asarray(inp["tgu_b1"])
    sg = np.asarray(inp["tgu_sg"])
    sb = np.asarray(inp["tgu_sb"])
    cw = np.asarray(inp["tgu_cw"])[:, :, 0, :]  # [to, ti, dx]
    cb = np.asarray(inp["tgu_cb"])
    w2 = np.asarray(inp["tgu_w2"])
    b2 = np.asarray(inp["tgu_b2"])
    w1p = np.ascontiguousarray((w1 * ng[None, :]).T).astype(f32)
    fp8_out["t_w18"] = _dr_pack_w1(w1p * W1_SCALE)
    b1f = (b1 + w1 @ nb).astype(f32)
    flags["t_b1"] = bool(np.any(b1f))
    c["t_b1"] = (b1f * W1_SCALE)[None, :]
    for dx in range(3):
        blk = np.zeros((128, 128), f32)
        lh = np.ascontiguousarray(cw[:, :, dx].T)  # [ti, to]
        blk[:64, :64] = lh
        blk[64:, 64:] = lh
        c[f"t_cw{dx}"] = blk
    # conv bias as a rank-4 matmul: cb + sum_dx cwsum_dx[to]*sb[fo+dx-1]
    cwsum = cw.sum(1)  # [to, dx]
    lhs = np.zeros((4, 128), f32)
    lhs[0] = np.concatenate([cb, cb])
    for dx in range(3):
        lhs[1 + dx] = np.concatenate([cwsum[:, dx], cwsum[:, dx]])
    rhs = np.zeros((4, 512), f32)
    rhs[0] = 1.0
    rhs[1, 1:] = sb[:511]   # dx=0 reads sb[fo-1]
    rhs[2] = sb             # dx=1 reads sb[fo]
    rhs[3, :511] = sb[1:]   # dx=2 reads sb[fo+1]
    c["t_cb_l"] = lhs
    c["t_cb_r"] = rhs
    c["t_w2r"] = np.ascontiguousarray(w2.T).astype(f32)
    flags["t_b2"] = bool(np.any(b2))
    c["t_b2"] = b2[None, :].astype(f32)
    wsl = w_out[:, 0:256]
    c["t_wpr"] = np.ascontiguousarray((wsl * np.asarray(inp["n1_g"])[None, :]).T).astype(f32)
    bout = bout + wsl @ np.asarray(inp["n1_b"])
    flags["t_sg"] = bool(np.any(sg != 1.0))
    c["t_sg"] = np.broadcast_to(sg, (128, 512)).astype(f32).copy()
    flags["bout"] = bool(np.any(bout))
    c["bout"] = bout[None, :].astype(f32)

    out = {}
    for k, v in c.items():
        if k.startswith("eps_col"):
            out[k] = np.ascontiguousarray(v, dtype=f32)
        else:
            out[k] = np.ascontiguousarray(v).astype(bf)
    for k, v in fp8_out.items():
        out[k] = _fp8(v)
    return out, flags


# Constant tensors DMA'd to SBUF once.
_WSHAPES = {
    "a_hat": ([128, 128], BF16), "ident": ([128, 128], BF16),
    "ones_row": ([1, 128], BF16), "eps_col": ([128, 1], F32),
    "eps_col_f": ([128, 1], F32),
    "t_w18": ([128, 2048], FP8), "t_b1": ([1, 1024], BF16),
    "t_cw0": ([128, 128], BF16), "t_cw1": ([128, 128], BF16),
    "t_cw2": ([128, 128], BF16),
    "t_cb_l": ([4, 128], BF16), "t_cb_r": ([4, 512], BF16),
    "t_w2r": ([512, 256], BF16), "t_b2": ([1, 256], BF16),
    "t_wpr": ([256, 256], BF16),
    "t_sg": ([128, 512], BF16), "bout": ([1, 256], BF16),
}
for _p in ("s", "c"):
    _WSHAPES.update({
        f"{_p}_w18": ([128, 2048], FP8), f"{_p}_b1": ([1, 1024], BF16),
        f"{_p}_grhs8": ([128, 2048], FP8),
        f"{_p}_bg_l": ([2, 128], BF16), f"{_p}_bg_r": ([2, 512], BF16),
        f"{_p}_w2r": ([512, 256], BF16), f"{_p}_b2": ([1, 256], BF16),
        f"{_p}_wpr": ([256, 256], BF16),
    })

DR = mybir.MatmulPerfMode.DoubleRow


def _emit(nc, n_gcn, n_conv, flags):
    xg = nc.dram_tensor("xg", [n_gcn * 128, 256], F32, kind="ExternalInput")
    xc = nc.dram_tensor("xc", [n_conv * 128, 256], F32, kind="ExternalInput")
    og = nc.dram_tensor("og", [n_gcn * 128, 256], F32, kind="ExternalOutput")
    oc = nc.dram_tensor("oc", [n_conv * 128, 256], F32, kind="ExternalOutput")

    wd = {
        k: nc.dram_tensor(k, shp, dt, kind="ExternalInput")
        for k, (shp, dt) in _WSHAPES.items()
    }

    with tile.TileContext(nc) as tc, ExitStack() as ctx:
        cpool = ctx.enter_context(tc.tile_pool(name="consts", bufs=1))
        wp = ctx.enter_context(tc.tile_pool(name="work", bufs=int(os.environ.get("WP_BUFS", "4"))))
        wpl = ctx.enter_context(tc.tile_pool(name="workl", bufs=int(os.environ.get("WPL_BUFS", "10"))))
        sp = ctx.enter_context(tc.tile_pool(name="stats", bufs=int(os.environ.get("SP_BUFS", "8"))))
        pp = ctx.enter_context(tc.tile_pool(name="ps", bufs=int(os.environ.get("FIN_BUFS", "1")), space="PSUM"))
        ppa = ctx.enter_context(tc.tile_pool(name="psa", bufs=int(os.environ.get("GW_BUFS", "2")), space="PSUM"))
        ppt = ctx.enter_context(tc.tile_pool(name="pst", bufs=int(os.environ.get("TP_BUFS", "2")), space="PSUM"))
        pps = ctx.enter_context(tc.tile_pool(name="pss", bufs=int(os.environ.get("PSS_BUFS", "1")), space="PSUM"))
        pph = ctx.enter_context(tc.tile_pool(name="psh", bufs=int(os.environ.get("HP_BUFS", "1")), space="PSUM"))

        GRP = int(os.environ.get("GRP", "8"))
        SC_APPLY = os.environ.get("SC_APPLY", "0") == "1"

        # Const loads: order by first use; round-robin dispatch queues.
        _prio = ["eps_col", "ident", "ones_row", "s_w18", "s_b1", "c_w18",
                 "c_b1", "a_hat", "s_grhs8", "s_bg_l", "s_bg_r", "s_w2r",
                 "s_b2", "c_grhs8", "c_bg_l", "c_bg_r", "c_w2r", "c_b2",
                 "s_wpr", "c_wpr", "bout", "t_w18", "t_b1", "t_sg", "t_cw0",
                 "t_cw1", "t_cw2", "t_cb_l", "t_cb_r", "t_w2r", "t_b2",
                 "t_wpr"]
        _qs = [nc.sync, nc.scalar, nc.gpsimd]
        _qi = [0]

        def _cdma(dst, src):
            _qs[_qi[0] % len(_qs)].dma_start(dst, src)
            _qi[0] += 1

        CS = {}

        def _load_const(name):
            d = wd[name]
            dt = d.dtype
            pdim = d.shape[0]
            if pdim <= 128:
                t = cpool.tile(list(d.shape), dt, tag=name)
                _cdma(t[:], d[:])
                CS[name] = t
            else:
                ts = []
                for i in range(pdim // 128):
                    t = cpool.tile([128, d.shape[1]], dt, tag=f"{name}{i}")
                    _cdma(t[:], d[i * 128 : (i + 1) * 128, :])
                    ts.append(t)
                CS[name] = ts

        _load_const("eps_col")
        _load_const("eps_col_f")
        _load_const("ident")

        def ln_pre(xin, vc, i, tag):
            """bn_stats + bn_aggr into columns [2i, 2i+1] of the group's
            shared (mean, var) tile vc."""
            s6 = sp.tile([128, 6], F32, tag=f"s6{tag}")
            nc.vector.bn_stats(s6[:], xin)
            nc.vector.bn_aggr(vc[:, 2 * i : 2 * i + 2], s6[:])

        def ln_finish(vc, k, tag, fin_scaled=False):
            """One sqrt + one reciprocal over the whole group's stat tile.
            Even columns hold means (sqrt/recip of those are garbage but
            never read); odd columns become rstd. With fin_scaled, computes
            sqrt(S^2 var + S^2 eps) = S*std so rc = rstd/S (S=FIN_SCALE) --
            the 1/S that pairs with the x S baked into the fp8 wpr."""
            sc = sp.tile([128, 2 * k], F32, tag=f"sc{tag}")
            if fin_scaled:
                _chain(nc.scalar.activation(
                    sc[:], vc[:, 0 : 2 * k], AF.Sqrt,
                    bias=CS["eps_col_f"][:], scale=FIN_SCALE * FIN_SCALE))
            else:
                _chain(nc.scalar.activation(sc[:], vc[:, 0 : 2 * k], AF.Sqrt,
                                            bias=CS["eps_col"][:]))
            rc = sp.tile([128, 2 * k], F32, tag=f"rc{tag}")
            nc.vector.reciprocal(rc[:], sc[:])
            if not SC_APPLY:
                return rc, None
            vc3 = vc[:, 0 : 2 * k].rearrange("p (k two) -> p k two", two=2)
            rc3 = rc[:, 0 : 2 * k].rearrange("p (k two) -> p k two", two=2)
            mr = sp.tile([128, k], F32, tag=f"mr{tag}")
            nc.vector.tensor_tensor(mr[:], vc3[:, :, 0:1], rc3[:, :, 1:2],
                                    op=ALU.mult)
            nmr = sp.tile([128, k], F32, tag=f"nmr{tag}")
            nc.vector.tensor_scalar(nmr[:], mr[:], scalar1=-1.0, scalar2=None,
                                    op0=ALU.mult)
            return rc, nmr

        def ln_apply(xin, width, vc, rc, i, tag, eng=None, nmr=None):
            """bf16 (x - mean) * rstd."""
            pool = wpl if tag.startswith(("ln2", "ln3")) else wp
            out = pool.tile([128, width], BF16, tag=f"nrm{tag}", bufs=GRP + 2)
            if nmr is not None:
                # scalar engine: x*rstd + (-mean*rstd)
                nc.scalar.activation(
                    out[:], xin, AF.Identity,
                    bias=nmr[:, i : i + 1],
                    scale=rc[:, 2 * i + 1 : 2 * i + 2],
                )
            else:
                (eng or nc.vector).tensor_scalar(
                    out[:], xin, scalar1=vc[:, 2 * i : 2 * i + 1],
                    scalar2=rc[:, 2 * i + 1 : 2 * i + 2],
                    op0=ALU.subtract, op1=ALU.mult,
                )
            return out

        def pe_transpose(xin, width, tag, out_dt=BF16, evac=None):
            """PE identity transpose + engine evacuation (cast on write).
            GpSimd can't touch PSUM, so evac is scalar (default) or DVE."""
            out = (wpl if tag == "xhT" else wp).tile(
                [128, width], out_dt, tag=tag,
                bufs=(GRP + 2) if tag == "xhT" else None)
            tps = ppt.tile([128, 512], BF16, tag="tpose")
            for cc in range(width // 128):
                sl = slice(cc * 128, (cc + 1) * 128)
                nc.tensor.transpose(tps[:, sl], xin[:, sl], CS["ident"][:])
            (evac or nc.scalar.copy)(out[:], tps[:, 0:width])
            return out

        def dma_transpose(xin, width, tag, q="sync"):
            out = (wpl if tag.startswith("xhT") else wp).tile(
                [128, width], BF16, tag=tag,
                bufs=(GRP + 2) if tag.startswith("xhT") else None)
            o3 = out[:].rearrange("p (c q) -> p c q", q=128)
            getattr(nc, q).dma_start_transpose(o3, xin[:])
            return out

        def mlp_uv(xhT8, w18, b1, has_b1, tag):
            """fp8 DoubleRow x̂ @ w1p (x W1_SCALE) [+ b1], half at a time
            into separate PSUM banks, gelu'd per half (v first: LN2 needs
            it immediately; u only gates later). -> (u, v) bf16 SBUF."""
            lhs3 = xhT8[:].rearrange("p (k m) -> p k m", k=2)
            out = {}
            for j in (1, 0):  # v-half first
                h_ps = pph.tile([128, 512], F32, tag=("vps" if j else "ups"))
                w3 = w18[:, j * 1024 : (j + 1) * 1024].rearrange(
                    "p (k f) -> p k f", k=2)
                nc.tensor.matmul(
                    h_ps[:], lhs3, w3, start=True, stop=not has_b1,
                    perf_mode=DR,
                )
                if has_b1:
                    nc.tensor.matmul(
                        h_ps[:], CS["ones_row"][:], b1[:, j * 512 : (j + 1) * 512],
                        start=False, stop=True, skip_group_check=True,
                    )
                hv = wpl.tile([128, 512], BF16, tag=("v" if j else "u") + tag,
                              bufs=GRP + 2)
                _chain(nc.scalar.activation(hv[:], h_ps[:], AF.Gelu,
                                            scale=1.0 / W1_SCALE))
                out["v" if j else "u"] = hv
            return out["u"], out["v"]

        GT_DMA = os.environ.get("GT_DMA", "1") == "1"

        def backend_mid(gated_src, u, X, p):
            """gated = psum*u, transpose, w2 matmul [+ b2] + residual -> blk."""
            gated = wp.tile([128, 512], BF16, tag="gated")
            nc.vector.tensor_tensor(gated[:], gated_src[:], u, op=ALU.mult)
            if GT_DMA:
                gT = dma_transpose(gated[:], 512, "gT")
            else:
                gT = pe_transpose(gated[:], 512, "gT")
            blk_ps = pps.tile([128, 256], F32, tag="smallps")
            has_b2 = flags[f"{p}_b2"]
            for fc in range(4):
                sl = slice(fc * 128, (fc + 1) * 128)
                nc.tensor.matmul(
                    blk_ps[:], gT[:, sl], CS[f"{p}_w2r"][fc][:],
                    start=(fc == 0), stop=(fc == 3 and not has_b2),
                )
            if has_b2:
                nc.tensor.matmul(
                    blk_ps[:], CS["ones_row"][:], CS[f"{p}_b2"][:],
                    start=False, stop=True, skip_group_check=True,
                )
            blk = wpl.tile([128, 256], BF16, tag=f"blk{p}", bufs=GRP + 1)
            nc.vector.tensor_tensor(blk[:], blk_ps[:], X[:], op=ALU.add)
            return blk

        # ---------------- software-pipelined tile stream ----------------
        # Tiles 0..n_gcn-1 are GCN (xg), the rest conv (xc). Processed in
        # groups of GRP with phase A (load+LN1+transpose) of group g emitted
        # before phases B (mlp+gelu) / C (mixers+backends) of group g-1, so
        # the scalar engine sees [sqrt block][gelu block] per group — 2
        # act-table trips per group instead of 2+ per tile. The _chain dep
        # forces that issue order (the tile scheduler is a greedy ready-heap
        # and would otherwise interleave).
        _last_act = [None]

        def _chain(bi):
            if _last_act[0] is not None:
                tile.add_dep_helper(bi.ins, _last_act[0].ins, reason="act-table order")
            _last_act[0] = bi
            return bi

        XHT_DMA = os.environ.get("XHT_DMA", "0") == "1"

        def phase_a_group(ts):
            states = []
            vc = sp.tile([128, 2 * len(ts)], F32, tag="vcln1")
            for i, t in enumerate(ts):
                xsrc, off = (xg, t) if t < n_gcn else (xc, t - n_gcn)
                X = wpl.tile([128, 256], F32, tag="X", bufs=2 * GRP + 2)
                _XQ = os.environ.get("XQ", "gpsimd")
                getattr(nc, _XQ).dma_start(X[:], xsrc[off * 128 : (off + 1) * 128, :])
                ln_pre(X[:], vc, i, "ln1")
                states.append({"t": t, "X": X})
            rc, nmr = ln_finish(vc, len(ts), "ln1")
            for i, st in enumerate(states):
                xhat = ln_apply(st["X"][:], 256, vc, rc, i, "ln1", nmr=nmr)
                if XHT_DMA:
                    xhT = dma_transpose(xhat[:], 256, "xhTb")
                    xhT8 = wpl.tile([128, 256], FP8, tag="xhT", bufs=GRP + 2)
                    nc.scalar.copy(xhT8[:], xhT[:])
                else:
                    xhT8 = pe_transpose(xhat[:], 256, "xhT", out_dt=FP8)
                st["xhT"] = xhT8
            return states

        def phase_b(st):
            if st["t"] < n_gcn:
                st["u"], st["v"] = {}, {}
                for p in ("s", "c"):
                    st["u"][p], st["v"][p] = mlp_uv(
                        st["xhT"], CS[f"{p}_w18"], CS[f"{p}_b1"][:],
                        flags[f"{p}_b1"], p)
            else:
                st["u"], st["v"] = mlp_uv(st["xhT"], CS["t_w18"],
                                          CS["t_b1"][:], flags["t_b1"], "s")

        def phase_c_group(states):
            # C-ln2: all LN2 stats of the group -> one sqrt + one recip
            n_ln2 = sum(2 if st["t"] < n_gcn else 1 for st in states)
            vc2 = sp.tile([128, 2 * n_ln2], F32, tag="vcln2")
            li = 0
            for st in states:
                if st["t"] < n_gcn:
                    for p in ("s", "c"):
                        ln_pre(st["v"][p][:], vc2, li, f"ln2{p}")
                        st[f"li_{p}"] = li
                        li += 1
                else:
                    ln_pre(st["v"][:], vc2, li, "ln2t")
                    st["li_t"] = li
                    li += 1
            rc2, _ = ln_finish(vc2, n_ln2, "ln2f")
            for st in states:
                if st["t"] < n_gcn:
                    st["vhat"] = {
                        p: ln_apply(st["v"][p][:], 512, vc2, rc2,
                                    st[f"li_{p}"], f"ln2{p}")
                        for p in ("s", "c")
                    }
                else:
                    st["vhat"] = ln_apply(st["v"][:], 512, vc2, rc2,
                                          st["li_t"], "ln2t")
            # C-mid: mixers + w2 + residual (no table-sensitive scalar ops)
            for st in states:
                t, X = st["t"], st["X"]
                st["blk"] = {}
                if t < n_gcn:
                    for p in ("s", "c"):
                        u = st["u"][p][:]
                        vhat = st["vhat"][p]
                        yt_ps = ppa.tile([128, 512], F32, tag="gwork")
                        for fc in range(4):
                            sl = slice(fc * 128, (fc + 1) * 128)
                            nc.tensor.matmul(
                                yt_ps[:, sl], vhat[:, sl], CS["a_hat"][:],
                                start=True, stop=True,
                            )
                        yt = wp.tile([128, 512], FP8, tag="yt")
                        nc.scalar.copy(yt[:], yt_ps[:])
                        g_ps = ppa.tile([128, 512], F32, tag="gwork")
                        for j in range(2):
                            lhs3 = yt[:, j * 256 : (j + 1) * 256].rearrange(
                                "p (k m) -> p k m", k=2)
                            g3 = CS[f"{p}_grhs8"][:, j * 1024 : (j + 1) * 1024].rearrange(
                                "p (k f) -> p k f", k=2)
                            nc.tensor.matmul(
                                g_ps[:], lhs3, g3,
                                start=(j == 0), stop=False, perf_mode=DR,
                                skip_group_check=(j == 1),
                            )
                        nc.tensor.matmul(
                            g_ps[:], CS[f"{p}_bg_l"][:], CS[f"{p}_bg_r"][:],
                            start=False, stop=True, skip_group_check=True,
                        )
                        st["blk"][p] = backend_mid(g_ps, u, X, p)
                else:
                    u = st["u"][:]
                    if flags["t_sg"]:
                        vs = wp.tile([128, 512], BF16, tag="vs")
                        nc.vector.tensor_tensor(vs[:], st["vhat"][:], CS["t_sg"][:], op=ALU.mult)
                    else:
                        vs = st["vhat"]
                    gc_ps = ppa.tile([128, 512], F32, tag="gwork")
                    nc.tensor.matmul(gc_ps[:, 0:512], CS["t_cw1"][:], vs[:, 0:512],
                                     start=True, stop=False)
                    nc.tensor.matmul(gc_ps[:, 1:512], CS["t_cw0"][:], vs[:, 0:511],
                                     start=False, stop=False, skip_group_check=True)
                    nc.tensor.matmul(gc_ps[:, 0:511], CS["t_cw2"][:], vs[:, 1:512],
                                     start=False, stop=False, skip_group_check=True)
                    nc.tensor.matmul(gc_ps[:, 0:512], CS["t_cb_l"][:], CS["t_cb_r"][:],
                                     start=False, stop=True, skip_group_check=True)
                    st["blk"]["t"] = backend_mid(gc_ps, u, X, "t")
            # C-ln3: all LN3 stats -> one sqrt + one recip
            n_ln3 = sum(len(st["blk"]) for st in states)
            vc3 = sp.tile([128, 2 * n_ln3], F32, tag="vcln3")
            li = 0
            for st in states:
                for p, blk in st["blk"].items():
                    ln_pre(blk[:], vc3, li, f"ln3{p}")
                    st[f"l3_{p}"] = li
                    li += 1
            rc3, nmr3 = ln_finish(vc3, n_ln3, "ln3")
            for st in states:
                st["xsh"] = {}
                for p, blk in st["blk"].items():
                    st["xsh"][p] = ln_apply(blk[:], 256, vc3, rc3,
                                            st[f"l3_{p}"], f"ln3{p}", nmr=nmr3)
            # C-fin: final projections + residual + stores
            for st in states:
                t, X = st["t"], st["X"]
                if t < n_gcn:
                    fin_ps = pp.tile([128, 256], F32, tag="finps")
                    for bi, p in enumerate(("s", "c")):
                        xshT = pe_transpose(st["xsh"][p][:], 256, "xshT",
                                            evac=nc.scalar.copy)
                        for cc in range(2):
                            sl = slice(cc * 128, (cc + 1) * 128)
                            nc.tensor.matmul(
                                fin_ps[:], xshT[:, sl], CS[f"{p}_wpr"][cc][:],
                                start=(bi == 0 and cc == 0),
                                stop=(bi == 1 and cc == 1 and not flags["bout"]),
                                skip_group_check=True,
                            )
                    if flags["bout"]:
                        nc.tensor.matmul(
                            fin_ps[:], CS["ones_row"][:], CS["bout"][:],
                            start=False, stop=True, skip_group_check=True,
                        )
                    outt = wp.tile([128, 256], F32, tag="outt")
                    nc.vector.tensor_tensor(outt[:], fin_ps[:], X[:], op=ALU.add)
                    getattr(nc, os.environ.get("OQ", "gpsimd")).dma_start(og[t * 128 : (t + 1) * 128, :], outt[:])
                else:
                    i = t - n_gcn
                    xshT = pe_transpose(st["xsh"]["t"][:], 256, "xshT",
                                        evac=nc.scalar.copy)
                    oc_ps = pps.tile([128, 256], F32, tag="smallps")
                    for cc in range(2):
                        sl = slice(cc * 128, (cc + 1) * 128)
                        nc.tensor.matmul(
                            oc_ps[:], xshT[:, sl], CS["t_wpr"][cc][:],
                            start=(cc == 0), stop=(cc == 1), skip_group_check=True,
                        )
                    occ = wp.tile([128, 256], F32, tag="outt")
                    nc.scalar.copy(occ[:], oc_ps[:])
                    getattr(nc, os.environ.get("OQ", "gpsimd")).dma_start(oc[i * 128 : (i + 1) * 128, :], occ[:])

        n_tiles = n_gcn + n_conv
        if os.environ.get("TILE_ORDER", "seq") == "mix" and n_gcn == n_conv:
            order = [t for i in range(n_gcn) for t in (i, n_gcn + i)]
        else:
            order = list(range(n_tiles))
        pending = phase_a_group([order[t] for t in range(0, min(GRP, n_tiles))])
        for name in _prio:
            if name not in CS:
                _load_const(name)
        for g0 in range(GRP, n_tiles, GRP):
            cur = phase_a_group([order[t] for t in range(g0, min(g0 + GRP, n_tiles))])
            for st in pending:
                phase_b(st)
            phase_c_group(pending)
            pending = cur
        for st in pending:
            phase_b(st)
        phase_c_group(pending)


def build(flags, n_gcn=N_GCN, n_conv=N_CONV):
    nc = bass.Bass()
    _emit(nc, n_gcn, n_conv, flags)
    _split_multi_waits(nc)
    return nc


def kernel(**inputs):
    consts, flags = _host_prep(inputs)
    x = np.ascontiguousarray(np.asarray(inputs["x"], dtype=np.float32))
    xg_full = x.reshape(B * T, N, D)
    xc_full = np.ascontiguousarray(x.transpose(0, 2, 1, 3)).reshape(B * N, T, D)

    nc = build(flags)
    in_maps = []
    for k in range(NCORES):
        m = dict(consts)
        m["xg"] = np.ascontiguousarray(xg_full[32 * k : 32 * (k + 1)]).reshape(N_GCN * 128, 256)
        m["xc"] = np.ascontiguousarray(xc_full[64 * k : 64 * (k + 1)]).reshape(N_CONV * 128, 256)
        in_maps.append(m)
    trace = os.environ.get("BASS_KERNEL_TRACE") == "1"
    res = run_bass_kernel_spmd(nc, in_maps, core_ids=list(range(NCORES)), trace=trace)
    if trace and res.exec_time_ns is not None:
        print(f"HW exec time: {res.exec_time_ns} ns")
    kernel.last_result = res
    og_full = np.stack([r["og"] for r in res.results]).reshape(B * T, N, D).reshape(B, T, N, D)
    oc_full = (
        np.stack([r["oc"] for r in res.results])
        .reshape(B * N, T, D)
        .reshape(B, N, T, D)
        .transpose(0, 2, 1, 3)
    )
    return (og_full + oc_full).astype(np.float32)


# revision 34
# speedup vs baseline: 1.2255x; 1.0127x over previous
"""Trainium2 Bass kernel for nn_Cy2Mixer_layer (gMLP block with conv/GCN/GCN
spatial mixers + fused output projection).

Sharding (8 cores):
  - The two GCN branches (sgu, cgu) + final projection/bias/residual are
    data-parallel over (B*T): 256 token-groups -> 32 per core, each a
    [N=128, D=256] tile (tokens on partitions).
  - The conv branch (tgu, Conv2d(T,T,(1,3)) channel mixer) needs full T per
    (b, n), so it is data-parallel over (B*N): 512 rows -> 64 per core,
    processed as 32 tiles of 2 rows ([2*T=128, D=256], tokens on partitions;
    the T-channel mix is a block-diagonal [128,128] matmul).
  Core outputs: og = xs/xc projections + b_out + residual (bt-sharded) and
  oc = xt projection (bn-sharded); the host scatters and adds the two.

Optimizations over the bf16 baseline (957us -> ~633us on-core):
  - w1 and gcn-lin matmuls run as fp8e4m3 DoubleRow (2 stacked K-tiles per
    pass). Host-side power-of-2 scales put the 0.02-magnitude weights into
    fp8's normal range; the scales fold out for free downstream (gelu's
    input scale for w1, the pre-divided w2 for gcn-lin).
  - Bias matmuls whose vectors are exactly zero for the given inputs are
    not emitted (b1/b2/b_out are zeros per the problem spec).
  - The per-group LN sqrt/reciprocal are batched into single wide
    instructions over a shared (mean,var)-column tile: 2 act-table loads
    per pipeline group instead of 2 per tile.
  - gelu is split into per-half ACTIVATEs over separate u/v PSUM banks
    (v first: LN2 consumes it immediately; u only gates later) so the next
    tile's w1 matmuls restart half a gelu earlier.
  - x-hat/xsh transposes on the PE (identity matmul) with scalar-engine
    PSUM evacuation (casting to fp8 on the way out); the gated transpose
    stays on the DMA XBAR via the sync queue; input loads and output
    stores dispatch from the GpSimd SWDGE queue. GpSimd cannot touch PSUM
    and its Q7 tensor ops are ~4us per 256-wide tile -- keep real
    elementwise work on DVE/scalar.
  - Software pipeline of GRP=10 tiles with per-tag SBUF ring depths.
"""

import os
import sys
from contextlib import ExitStack

for _p in ("/opt/trn_rl_repo", "/root/.axon_site/_ro/trn_rl_repo"):
    if os.path.isdir(_p) and _p not in sys.path:
        sys.path.insert(0, _p)

import numpy as np

import bass_rust
import concourse.bass as bass
import concourse.tile as tile
from concourse import mybir
from concourse.bass_utils import run_bass_kernel_spmd

if os.environ.get("LDW_OPT") == "1":
    from concourse import bass_utils as _bu
    _orig_run_command = _bu.run_command

    def _run_command_ldw(cmd, **kw):
        cmd = ["--enable-ldw-opt=true" if c == "--enable-ldw-opt=false" else c
               for c in cmd]
        return _orig_run_command(cmd, **kw)

    _bu.run_command = _run_command_ldw

AF = mybir.ActivationFunctionType
ALU = mybir.AluOpType
F32 = mybir.dt.float32
BF16 = mybir.dt.bfloat16
FP8 = mybir.dt.float8e4
LN_EPS = 1e-5

B, T, N, D, F = 4, 64, 128, 256, 512
NCORES = 8
N_GCN = 32   # bt tiles per core
N_CONV = 32  # conv tiles per core (2 bn rows each)

W1_SCALE = 64.0   # folded out via gelu's input scale
GR_SCALE = 8.0    # folded out via pre-divided w2
FIN_SCALE = 4.0   # split scale: xsh carries 1/4 (via rstd), wpr carries x4

_ctr = [0]


def _split_multi_waits(nc):
    """This walrus build rejects any instruction carrying >1 sync wait
    ("Too many sync wait commands"). Hoist all-but-one wait of every
    instruction onto dedicated same-engine NOPs inserted before it."""
    for f in nc.m.functions:
        for bb in f.blocks:
            insts = bb.instructions
            i = 0
            while i < len(insts):
                inst = insts[i]
                si = inst.sync_info
                if si is not None and si.on_wait is not None and len(si.on_wait) > 1:
                    waits = list(si.on_wait)
                    upd = list(si.on_update) if si.on_update is not None else []
                    inst.sync_info = bass_rust.SyncInfo(
                        on_wait=[waits[-1]], on_update=upd
                    )
                    for w in waits[:-1]:
                        _ctr[0] += 1
                        nop = mybir.InstNoOp(
                            name=f"wsplit-{_ctr[0]}",
                            sync_info=mybir.SyncInfo(on_wait=[w], on_update=[]),
                            bass_nofuse=True,
                            engine=inst.engine,
                        )
                        insts.insert(i, nop)
                        i += 1
                i += 1


def _fp8(x):
    dt = np.dtype(mybir.dt.np(FP8))
    return np.clip(np.asarray(x, np.float32), -240.0, 240.0).astype(dt)


def _dr_pack(w):
    """[K=256 or 512, Fout] -> fp8 DoubleRow layout [128, J, 2, Fout] flattened
    to [128, J*2*Fout] where pair j covers K-chunks (2j, 2j+1)."""
    K, Fo = w.shape
    nj = K // 256
    out = np.zeros((128, nj, 2, Fo), np.float32)
    for j in range(nj):
        for k in range(2):
            out[:, j, k, :] = w[(2 * j + k) * 128 : (2 * j + k + 1) * 128, :]
    return out.reshape(128, nj * 2 * Fo)


def _dr_pack_w1(w1p):
    """[D=256, 2F=1024] -> [128, 2048] with layout [p, (half, k, f512)]:
    half j selects the output 512-block, k the D-chunk of the contraction."""
    return np.concatenate(
        [_dr_pack(w1p[:, 0:512]), _dr_pack(w1p[:, 512:1024])], axis=1
    )


def _host_prep(inp):
    """Fold LN affines into weights; build matmul-ready constant layouts."""
    f32 = np.float32
    bf = np.dtype(mybir.dt.np(BF16))
    c = {}
    flags = {}
    cir = np.asarray(inp["cirmat"])
    a = (cir != 0).astype(f32)
    np.fill_diagonal(a, 1.0)
    deg = a.sum(0).astype(f32)
    dinv = (1.0 / np.sqrt(deg)).astype(f32)
    a_hat = (a * dinv[:, None] * dinv[None, :]).astype(f32)
    c["a_hat"] = a_hat
    colsum = a_hat.sum(0).astype(f32)

    c["ident"] = np.eye(128, dtype=f32)
    c["eps_col"] = np.full((128, 1), LN_EPS, f32)
    c["eps_col_f"] = np.full((128, 1), LN_EPS * FIN_SCALE * FIN_SCALE, f32)
    c["ones_row"] = np.ones((1, 128), f32)

    w_out = np.asarray(inp["w_out"])
    bout = np.asarray(inp["b_out"]).astype(f32).copy()

    fp8_out = {}

    for p, pre, ng_, nb_, off in (
        ("s", "sgu", "n2_g", "n2_b", 256),
        ("c", "cgu", "n3_g", "n3_b", 512),
    ):
        ng = np.asarray(inp[f"{pre}_ng"])
        nb = np.asarray(inp[f"{pre}_nb"])
        w1 = np.asarray(inp[f"{pre}_w1"])
        b1 = np.asarray(inp[f"{pre}_b1"])
        sg = np.asarray(inp[f"{pre}_sg"])
        sb = np.asarray(inp[f"{pre}_sb"])
        gw = np.asarray(inp[f"{pre}_gw"])
        gb = np.asarray(inp[f"{pre}_gb"])
        w2 = np.asarray(inp[f"{pre}_w2"])
        b2 = np.asarray(inp[f"{pre}_b2"])
        w1p = np.ascontiguousarray((w1 * ng[None, :]).T).astype(f32)  # [D, 2F]
        fp8_out[f"{p}_w18"] = _dr_pack_w1(w1p * W1_SCALE)
        b1f = (b1 + w1 @ nb).astype(f32)
        flags[f"{p}_b1"] = bool(np.any(b1f))
        c[f"{p}_b1"] = (b1f * W1_SCALE)[None, :]
        grhs = np.ascontiguousarray((gw * sg[None, :]).T).astype(f32)  # [F, F]
        fp8_out[f"{p}_grhs8"] = _dr_pack(grhs * GR_SCALE)
        c[f"{p}_bg_l"] = np.stack([colsum, np.ones(128, f32)]).astype(f32)
        c[f"{p}_bg_r"] = (np.stack([gw @ sb, gb]) * GR_SCALE).astype(f32)
        c[f"{p}_w2r"] = np.ascontiguousarray(w2.T).astype(f32) / GR_SCALE
        flags[f"{p}_b2"] = bool(np.any(b2))
        c[f"{p}_b2"] = b2[None, :].astype(f32)
        wsl = w_out[:, off : off + 256]
        c[f"{p}_wpr"] = np.ascontiguousarray((wsl * np.asarray(inp[ng_])[None, :]).T).astype(f32)
        bout = bout + wsl @ np.asarray(inp[nb_])

    ng = np.asarray(inp["tgu_ng"])
    nb = np.asarray(inp["tgu_nb"])
    w1 = np.asarray(inp["tgu_w1"])
    b1 = np.asarray(inp["tgu_b1"])
    sg = np.asarray(inp["tgu_sg"])
    sb = np.asarray(inp["tgu_sb"])
    cw = np.asarray(inp["tgu_cw"])[:, :, 0, :]  # [to, ti, dx]
    cb = np.asarray(inp["tgu_cb"])
    w2 = np.asarray(inp["tgu_w2"])
    b2 = np.asarray(inp["tgu_b2"])
    w1p = np.ascontiguousarray((w1 * ng[None, :]).T).astype(f32)
    fp8_out["t_w18"] = _dr_pack_w1(w1p * W1_SCALE)
    b1f = (b1 + w1 @ nb).astype(f32)
    flags["t_b1"] = bool(np.any(b1f))
    c["t_b1"] = (b1f * W1_SCALE)[None, :]
    for dx in range(3):
        blk = np.zeros((128, 128), f32)
        lh = np.ascontiguousarray(cw[:, :, dx].T)  # [ti, to]
        blk[:64, :64] = lh
        blk[64:, 64:] = lh
        c[f"t_cw{dx}"] = blk
    # conv bias as a rank-4 matmul: cb + sum_dx cwsum_dx[to]*sb[fo+dx-1]
    cwsum = cw.sum(1)  # [to, dx]
    lhs = np.zeros((4, 128), f32)
    lhs[0] = np.concatenate([cb, cb])
    for dx in range(3):
        lhs[1 + dx] = np.concatenate([cwsum[:, dx], cwsum[:, dx]])
    rhs = np.zeros((4, 512), f32)
    rhs[0] = 1.0
    rhs[1, 1:] = sb[:511]   # dx=0 reads sb[fo-1]
    rhs[2] = sb             # dx=1 reads sb[fo]
    rhs[3, :511] = sb[1:]   # dx=2 reads sb[fo+1]
    c["t_cb_l"] = lhs
    c["t_cb_r"] = rhs
    c["t_w2r"] = np.ascontiguousarray(w2.T).astype(f32)
    flags["t_b2"] = bool(np.any(b2))
    c["t_b2"] = b2[None, :].astype(f32)
    wsl = w_out[:, 0:256]
    c["t_wpr"] = np.ascontiguousarray((wsl * np.asarray(inp["n1_g"])[None, :]).T).astype(f32)
    bout = bout + wsl @ np.asarray(inp["n1_b"])
    flags["t_sg"] = bool(np.any(sg != 1.0))
    c["t_sg"] = np.broadcast_to(sg, (128, 512)).astype(f32).copy()
    flags["bout"] = bool(np.any(bout))
    c["bout"] = bout[None, :].astype(f32)

    out = {}
    for k, v in c.items():
        if k.startswith("eps_col"):
            out[k] = np.ascontiguousarray(v, dtype=f32)
        else:
            out[k] = np.ascontiguousarray(v).astype(bf)
    for k, v in fp8_out.items():
        out[k] = _fp8(v)
    return out, flags


# Constant tensors DMA'd to SBUF once.
_WSHAPES = {
    "a_hat": ([128, 128], BF16), "ident": ([128, 128], BF16),
    "ones_row": ([1, 128], BF16), "eps_col": ([128, 1], F32),
    "eps_col_f": ([128, 1], F32),
    "t_w18": ([128, 2048], FP8), "t_b1": ([1, 1024], BF16),
    "t_cw0": ([128, 128], BF16), "t_cw1": ([128, 128], BF16),
    "t_cw2": ([128, 128], BF16),
    "t_cb_l": ([4, 128], BF16), "t_cb_r": ([4, 512], BF16),
    "t_w2r": ([512, 256], BF16), "t_b2": ([1, 256], BF16),
    "t_wpr": ([256, 256], BF16),
    "t_sg": ([128, 512], BF16), "bout": ([1, 256], BF16),
}
for _p in ("s", "c"):
    _WSHAPES.update({
        f"{_p}_w18": ([128, 2048], FP8), f"{_p}_b1": ([1, 1024], BF16),
        f"{_p}_grhs8": ([128, 2048], FP8),
        f"{_p}_bg_l": ([2, 128], BF16), f"{_p}_bg_r": ([2, 512], BF16),
        f"{_p}_w2r": ([512, 256], BF16), f"{_p}_b2": ([1, 256], BF16),
        f"{_p}_wpr": ([256, 256], BF16),
    })

DR = mybir.MatmulPerfMode.DoubleRow


def _emit(nc, n_gcn, n_conv, flags):
    xg = nc.dram_tensor("xg", [n_gcn * 128, 256], F32, kind="ExternalInput")
    xc = nc.dram_tensor("xc", [n_conv * 128, 256], F32, kind="ExternalInput")
    og = nc.dram_tensor("og", [n_gcn * 128, 256], F32, kind="ExternalOutput")
    oc = nc.dram_tensor("oc", [n_conv * 128, 256], F32, kind="ExternalOutput")

    wd = {
        k: nc.dram_tensor(k, shp, dt, kind="ExternalInput")
        for k, (shp, dt) in _WSHAPES.items()
    }

    with tile.TileContext(nc) as tc, ExitStack() as ctx:
        cpool = ctx.enter_context(tc.tile_pool(name="consts", bufs=1))
        wp = ctx.enter_context(tc.tile_pool(name="work", bufs=int(os.environ.get("WP_BUFS", "4"))))
        wpl = ctx.enter_context(tc.tile_pool(name="workl", bufs=int(os.environ.get("WPL_BUFS", "10"))))
        sp = ctx.enter_context(tc.tile_pool(name="stats", bufs=int(os.environ.get("SP_BUFS", "4"))))
        pp = ctx.enter_context(tc.tile_pool(name="ps", bufs=int(os.environ.get("FIN_BUFS", "1")), space="PSUM"))
        ppa = ctx.enter_context(tc.tile_pool(name="psa", bufs=int(os.environ.get("GW_BUFS", "2")), space="PSUM"))
        ppt = ctx.enter_context(tc.tile_pool(name="pst", bufs=int(os.environ.get("TP_BUFS", "2")), space="PSUM"))
        pps = ctx.enter_context(tc.tile_pool(name="pss", bufs=int(os.environ.get("PSS_BUFS", "1")), space="PSUM"))
        pph = ctx.enter_context(tc.tile_pool(name="psh", bufs=int(os.environ.get("HP_BUFS", "1")), space="PSUM"))

        GRP = int(os.environ.get("GRP", "10"))
        SC_APPLY = os.environ.get("SC_APPLY", "0") == "1"

        # Const loads: order by first use; round-robin dispatch queues.
        _prio = ["eps_col", "ident", "ones_row", "s_w18", "s_b1", "c_w18",
                 "c_b1", "a_hat", "s_grhs8", "s_bg_l", "s_bg_r", "s_w2r",
                 "s_b2", "c_grhs8", "c_bg_l", "c_bg_r", "c_w2r", "c_b2",
                 "s_wpr", "c_wpr", "bout", "t_w18", "t_b1", "t_sg", "t_cw0",
                 "t_cw1", "t_cw2", "t_cb_l", "t_cb_r", "t_w2r", "t_b2",
                 "t_wpr"]
        _qs = [nc.sync, nc.scalar, nc.gpsimd]
        _qi = [0]

        def _cdma(dst, src):
            _qs[_qi[0] % len(_qs)].dma_start(dst, src)
            _qi[0] += 1

        CS = {}

        def _load_const(name):
            d = wd[name]
            dt = d.dtype
            pdim = d.shape[0]
            if pdim <= 128:
                t = cpool.tile(list(d.shape), dt, tag=name)
                _cdma(t[:], d[:])
                CS[name] = t
            else:
                ts = []
                for i in range(pdim // 128):
                    t = cpool.tile([128, d.shape[1]], dt, tag=f"{name}{i}")
                    _cdma(t[:], d[i * 128 : (i + 1) * 128, :])
                    ts.append(t)
                CS[name] = ts

        _load_const("eps_col")
        _load_const("eps_col_f")
        _load_const("ident")

        def ln_pre(xin, vc, i, tag):
            """bn_stats + bn_aggr into columns [2i, 2i+1] of the group's
            shared (mean, var) tile vc."""
            s6 = sp.tile([128, 6], F32, tag=f"s6{tag}")
            nc.vector.bn_stats(s6[:], xin)
            nc.vector.bn_aggr(vc[:, 2 * i : 2 * i + 2], s6[:])

        def ln_finish(vc, k, tag, fin_scaled=False):
            """One sqrt + one reciprocal over the whole group's stat tile.
            Even columns hold means (sqrt/recip of those are garbage but
            never read); odd columns become rstd. With fin_scaled, computes
            sqrt(S^2 var + S^2 eps) = S*std so rc = rstd/S (S=FIN_SCALE) --
            the 1/S that pairs with the x S baked into the fp8 wpr."""
            sc = sp.tile([128, 2 * k], F32, tag=f"sc{tag}")
            if fin_scaled:
                _chain(nc.scalar.activation(
                    sc[:], vc[:, 0 : 2 * k], AF.Sqrt,
                    bias=CS["eps_col_f"][:], scale=FIN_SCALE * FIN_SCALE))
            else:
                _chain(nc.scalar.activation(sc[:], vc[:, 0 : 2 * k], AF.Sqrt,
                                            bias=CS["eps_col"][:]))
            rc = sp.tile([128, 2 * k], F32, tag=f"rc{tag}")
            nc.vector.reciprocal(rc[:], sc[:])
            if not SC_APPLY:
                return rc, None
            vc3 = vc[:, 0 : 2 * k].rearrange("p (k two) -> p k two", two=2)
            rc3 = rc[:, 0 : 2 * k].rearrange("p (k two) -> p k two", two=2)
            mr = sp.tile([128, k], F32, tag=f"mr{tag}")
            nc.vector.tensor_tensor(mr[:], vc3[:, :, 0:1], rc3[:, :, 1:2],
                                    op=ALU.mult)
            nmr = sp.tile([128, k], F32, tag=f"nmr{tag}")
            nc.vector.tensor_scalar(nmr[:], mr[:], scalar1=-1.0, scalar2=None,
                                    op0=ALU.mult)
            return rc, nmr

        def ln_apply(xin, width, vc, rc, i, tag, eng=None, nmr=None):
            """bf16 (x - mean) * rstd."""
            pool = wpl if tag.startswith(("ln2", "ln3")) else wp
            out = pool.tile([128, width], BF16, tag=f"nrm{tag}", bufs=GRP + 2)
            if nmr is not None:
                # scalar engine: x*rstd + (-mean*rstd)
                nc.scalar.activation(
                    out[:], xin, AF.Identity,
                    bias=nmr[:, i : i + 1],
                    scale=rc[:, 2 * i + 1 : 2 * i + 2],
                )
            else:
                (eng or nc.vector).tensor_scalar(
                    out[:], xin, scalar1=vc[:, 2 * i : 2 * i + 1],
                    scalar2=rc[:, 2 * i + 1 : 2 * i + 2],
                    op0=ALU.subtract, op1=ALU.mult,
                )
            return out

        def pe_transpose(xin, width, tag, out_dt=BF16, evac=None):
            """PE identity transpose + engine evacuation (cast on write).
            GpSimd can't touch PSUM, so evac is scalar (default) or DVE."""
            out = (wpl if tag == "xhT" else wp).tile(
                [128, width], out_dt, tag=tag,
                bufs=(GRP + 2) if tag == "xhT" else None)
            tps = ppt.tile([128, 512], BF16, tag="tpose")
            for cc in range(width // 128):
                sl = slice(cc * 128, (cc + 1) * 128)
                nc.tensor.transpose(tps[:, sl], xin[:, sl], CS["ident"][:])
            (evac or nc.scalar.copy)(out[:], tps[:, 0:width])
            return out

        def dma_transpose(xin, width, tag, q="sync"):
            out = (wpl if tag.startswith("xhT") else wp).tile(
                [128, width], BF16, tag=tag,
                bufs=(GRP + 2) if tag.startswith("xhT") else None)
            o3 = out[:].rearrange("p (c q) -> p c q", q=128)
            getattr(nc, q).dma_start_transpose(o3, xin[:])
            return out

        def mlp_uv(xhT8, w18, b1, has_b1, tag):
            """fp8 DoubleRow x̂ @ w1p (x W1_SCALE) [+ b1], half at a time
            into separate PSUM banks, gelu'd per half (v first: LN2 needs
            it immediately; u only gates later). -> (u, v) bf16 SBUF."""
            lhs3 = xhT8[:].rearrange("p (k m) -> p k m", k=2)
            out = {}
            for j in (1, 0):  # v-half first
                h_ps = pph.tile([128, 512], F32, tag=("vps" if j else "ups"))
                w3 = w18[:, j * 1024 : (j + 1) * 1024].rearrange(
                    "p (k f) -> p k f", k=2)
                nc.tensor.matmul(
                    h_ps[:], lhs3, w3, start=True, stop=not has_b1,
                    perf_mode=DR,
                )
                if has_b1:
                    nc.tensor.matmul(
                        h_ps[:], CS["ones_row"][:], b1[:, j * 512 : (j + 1) * 512],
                        start=False, stop=True, skip_group_check=True,
                    )
                hv = wpl.tile([128, 512], BF16, tag=("v" if j else "u") + tag,
                              bufs=GRP + 2)
                _chain(nc.scalar.activation(hv[:], h_ps[:], AF.Gelu,
                                            scale=1.0 / W1_SCALE))
                out["v" if j else "u"] = hv
            return out["u"], out["v"]

        GT_DMA = os.environ.get("GT_DMA", "1") == "1"

        def backend_mid(gated_src, u, X, p):
            """gated = psum*u, transpose, w2 matmul [+ b2] + residual -> blk."""
            gated = wp.tile([128, 512], BF16, tag="gated")
            nc.vector.tensor_tensor(gated[:], gated_src[:], u, op=ALU.mult)
            if GT_DMA:
                gT = dma_transpose(gated[:], 512, "gT")
            else:
                gT = pe_transpose(gated[:], 512, "gT")
            blk_ps = pps.tile([128, 256], F32, tag="smallps")
            has_b2 = flags[f"{p}_b2"]
            for fc in range(4):
                sl = slice(fc * 128, (fc + 1) * 128)
                nc.tensor.matmul(
                    blk_ps[:], gT[:, sl], CS[f"{p}_w2r"][fc][:],
                    start=(fc == 0), stop=(fc == 3 and not has_b2),
                )
            if has_b2:
                nc.tensor.matmul(
                    blk_ps[:], CS["ones_row"][:], CS[f"{p}_b2"][:],
                    start=False, stop=True, skip_group_check=True,
                )
            blk = wpl.tile([128, 256], BF16, tag=f"blk{p}", bufs=GRP + 1)
            nc.vector.tensor_tensor(blk[:], blk_ps[:], X[:], op=ALU.add)
            return blk

        # ---------------- software-pipelined tile stream ----------------
        # Tiles 0..n_gcn-1 are GCN (xg), the rest conv (xc). Processed in
        # groups of GRP with phase A (load+LN1+transpose) of group g emitted
        # before phases B (mlp+gelu) / C (mixers+backends) of group g-1, so
        # the scalar engine sees [sqrt block][gelu block] per group — 2
        # act-table trips per group instead of 2+ per tile. The _chain dep
        # forces that issue order (the tile scheduler is a greedy ready-heap
        # and would otherwise interleave).
        _last_act = [None]

        def _chain(bi):
            if _last_act[0] is not None:
                tile.add_dep_helper(bi.ins, _last_act[0].ins, reason="act-table order")
            _last_act[0] = bi
            return bi

        XHT_DMA = os.environ.get("XHT_DMA", "0") == "1"

        def phase_a_group(ts):
            states = []
            vc = sp.tile([128, 2 * len(ts)], F32, tag="vcln1")
            for i, t in enumerate(ts):
                xsrc, off = (xg, t) if t < n_gcn else (xc, t - n_gcn)
                X = wpl.tile([128, 256], F32, tag="X", bufs=2 * GRP + 2)
                _XQ = os.environ.get("XQ", "gpsimd")
                getattr(nc, _XQ).dma_start(X[:], xsrc[off * 128 : (off + 1) * 128, :])
                ln_pre(X[:], vc, i, "ln1")
                states.append({"t": t, "X": X})
            rc, nmr = ln_finish(vc, len(ts), "ln1")
            for i, st in enumerate(states):
                xhat = ln_apply(st["X"][:], 256, vc, rc, i, "ln1", nmr=nmr)
                if XHT_DMA:
                    xhT = dma_transpose(xhat[:], 256, "xhTb")
                    xhT8 = wpl.tile([128, 256], FP8, tag="xhT", bufs=GRP + 2)
                    nc.scalar.copy(xhT8[:], xhT[:])
                else:
                    xhT8 = pe_transpose(xhat[:], 256, "xhT", out_dt=FP8)
                st["xhT"] = xhT8
            return states

        def phase_b(st):
            if st["t"] < n_gcn:
                st["u"], st["v"] = {}, {}
                for p in ("s", "c"):
                    st["u"][p], st["v"][p] = mlp_uv(
                        st["xhT"], CS[f"{p}_w18"], CS[f"{p}_b1"][:],
                        flags[f"{p}_b1"], p)
            else:
                st["u"], st["v"] = mlp_uv(st["xhT"], CS["t_w18"],
                                          CS["t_b1"][:], flags["t_b1"], "s")

        def phase_c_group(states):
            # C-ln2: all LN2 stats of the group -> one sqrt + one recip
            n_ln2 = sum(2 if st["t"] < n_gcn else 1 for st in states)
            vc2 = sp.tile([128, 2 * n_ln2], F32, tag="vcln2")
            li = 0
            for st in states:
                if st["t"] < n_gcn:
                    for p in ("s", "c"):
                        ln_pre(st["v"][p][:], vc2, li, f"ln2{p}")
                        st[f"li_{p}"] = li
                        li += 1
                else:
                    ln_pre(st["v"][:], vc2, li, "ln2t")
                    st["li_t"] = li
                    li += 1
            rc2, _ = ln_finish(vc2, n_ln2, "ln2f")
            for st in states:
                if st["t"] < n_gcn:
                    st["vhat"] = {
                        p: ln_apply(st["v"][p][:], 512, vc2, rc2,
                                    st[f"li_{p}"], f"ln2{p}")
                        for p in ("s", "c")
                    }
                else:
                    st["vhat"] = ln_apply(st["v"][:], 512, vc2, rc2,
                                          st["li_t"], "ln2t")
            # C-mid: mixers + w2 + residual (no table-sensitive scalar ops)
            for st in states:
                t, X = st["t"], st["X"]
                st["blk"] = {}
                if t < n_gcn:
                    for p in ("s", "c"):
                        u = st["u"][p][:]
                        vhat = st["vhat"][p]
                        yt_ps = ppa.tile([128, 512], F32, tag="gwork")
                        for fc in range(4):
                            sl = slice(fc * 128, (fc + 1) * 128)
                            nc.tensor.matmul(
                                yt_ps[:, sl], vhat[:, sl], CS["a_hat"][:],
                                start=True, stop=True,
                            )
                        yt = wp.tile([128, 512], FP8, tag="yt")
                        nc.scalar.copy(yt[:], yt_ps[:])
                        g_ps = ppa.tile([128, 512], F32, tag="gwork")
                        for j in range(2):
                            lhs3 = yt[:, j * 256 : (j + 1) * 256].rearrange(
                                "p (k m) -> p k m", k=2)
                            g3 = CS[f"{p}_grhs8"][:, j * 1024 : (j + 1) * 1024].rearrange(
                                "p (k f) -> p k f", k=2)
                            nc.tensor.matmul(
                                g_ps[:], lhs3, g3,
                                start=(j == 0), stop=False, perf_mode=DR,
                                skip_group_check=(j == 1),
                            )
                        nc.tensor.matmul(
                            g_ps[:], CS[f"{p}_bg_l"][:], CS[f"{p}_bg_r"][:],
                            start=False, stop=True, skip_group_check=True,
                        )
                        st["blk"][p] = backend_mid(g_ps, u, X, p)
                else:
                    u = st["u"][:]
                    if flags["t_sg"]:
                        vs = wp.tile([128, 512], BF16, tag="vs")
                        nc.vector.tensor_tensor(vs[:], st["vhat"][:], CS["t_sg"][:], op=ALU.mult)
                    else:
                        vs = st["vhat"]
                    gc_ps = ppa.tile([128, 512], F32, tag="gwork")
                    nc.tensor.matmul(gc_ps[:, 0:512], CS["t_cw1"][:], vs[:, 0:512],
                                     start=True, stop=False)
                    nc.tensor.matmul(gc_ps[:, 1:512], CS["t_cw0"][:], vs[:, 0:511],
                                     start=False, stop=False, skip_group_check=True)
                    nc.tensor.matmul(gc_ps[:, 0:511], CS["t_cw2"][:], vs[:, 1:512],
                                     start=False, stop=False, skip_group_check=True)
                    nc.tensor.matmul(gc_ps[:, 0:512], CS["t_cb_l"][:], CS["t_cb_r"][:],
                                     start=False, stop=True, skip_group_check=True)
                    st["blk"]["t"] = backend_mid(gc_ps, u, X, "t")
            # C-ln3: all LN3 stats -> one sqrt + one recip
            n_ln3 = sum(len(st["blk"]) for st in states)
            vc3 = sp.tile([128, 2 * n_ln3], F32, tag="vcln3")
            li = 0
            for st in states:
                for p, blk in st["blk"].items():
                    ln_pre(blk[:], vc3, li, f"ln3{p}")
                    st[f"l3_{p}"] = li
                    li += 1
            rc3, nmr3 = ln_finish(vc3, n_ln3, "ln3")
            for st in states:
                st["xsh"] = {}
                for p, blk in st["blk"].items():
                    st["xsh"][p] = ln_apply(blk[:], 256, vc3, rc3,
                                            st[f"l3_{p}"], f"ln3{p}", nmr=nmr3)
            # C-fin: final projections + residual + stores
            for st in states:
                t, X = st["t"], st["X"]
                if t < n_gcn:
                    fin_ps = pp.tile([128, 256], F32, tag="finps")
                    for bi, p in enumerate(("s", "c")):
                        xshT = pe_transpose(st["xsh"][p][:], 256, "xshT",
                                            evac=nc.scalar.copy)
                        for cc in range(2):
                            sl = slice(cc * 128, (cc + 1) * 128)
                            nc.tensor.matmul(
                                fin_ps[:], xshT[:, sl], CS[f"{p}_wpr"][cc][:],
                                start=(bi == 0 and cc == 0),
                                stop=(bi == 1 and cc == 1 and not flags["bout"]),
                                skip_group_check=True,
                            )
                    if flags["bout"]:
                        nc.tensor.matmul(
                            fin_ps[:], CS["ones_row"][:], CS["bout"][:],
                            start=False, stop=True, skip_group_check=True,
                        )
                    outt = wp.tile([128, 256], F32, tag="outt")
                    nc.vector.tensor_tensor(outt[:], fin_ps[:], X[:], op=ALU.add)
                    getattr(nc, os.environ.get("OQ", "gpsimd")).dma_start(og[t * 128 : (t + 1) * 128, :], outt[:])
                else:
                    i = t - n_gcn
                    xshT = pe_transpose(st["xsh"]["t"][:], 256, "xshT",
                                        evac=nc.scalar.copy)
                    oc_ps = pps.tile([128, 256], F32, tag="smallps")
                    for cc in range(2):
                        sl = slice(cc * 128, (cc + 1) * 128)
                        nc.tensor.matmul(
                            oc_ps[:], xshT[:, sl], CS["t_wpr"][cc][:],
                            start=(cc == 0), stop=(cc == 1), skip_group_check=True,
                        )
                    occ = wp.tile([128, 256], F32, tag="outt")
                    nc.scalar.copy(occ[:], oc_ps[:])
                    getattr(nc, os.environ.get("OQ", "gpsimd")).dma_start(oc[i * 128 : (i + 1) * 128, :], occ[:])

        n_tiles = n_gcn + n_conv
        if os.environ.get("TILE_ORDER", "seq") == "mix" and n_gcn == n_conv:
            order = [t for i in range(n_gcn) for t in (i, n_gcn + i)]
        else:
            order = list(range(n_tiles))
        pending = phase_a_group([order[t] for t in range(0, min(GRP, n_tiles))])
        for name in _prio:
            if name not in CS:
                _load_const(name)
        for g0 in range(GRP, n_tiles, GRP):
            cur = phase_a_group([order[t] for t in range(g0, min(g0 + GRP, n_tiles))])
            for st in pending:
                phase_b(st)
            phase_c_group(pending)
            pending = cur
        for st in pending:
            phase_b(st)
        phase_c_group(pending)


def build(flags, n_gcn=N_GCN, n_conv=N_CONV):
    nc = bass.Bass()
    _emit(nc, n_gcn, n_conv, flags)
    _split_multi_waits(nc)
    return nc


def kernel(**inputs):
    consts, flags = _host_prep(inputs)
    x = np.ascontiguousarray(np.asarray(inputs["x"], dtype=np.float32))
    xg_full = x.reshape(B * T, N, D)
    xc_full = np.ascontiguousarray(x.transpose(0, 2, 1, 3)).reshape(B * N, T, D)

    nc = build(flags)
    in_maps = []
    for k in range(NCORES):
        m = dict(consts)
        m["xg"] = np.ascontiguousarray(xg_full[32 * k : 32 * (k + 1)]).reshape(N_GCN * 128, 256)
        m["xc"] = np.ascontiguousarray(xc_full[64 * k : 64 * (k + 1)]).reshape(N_CONV * 128, 256)
        in_maps.append(m)
    trace = os.environ.get("BASS_KERNEL_TRACE") == "1"
    res = run_bass_kernel_spmd(nc, in_maps, core_ids=list(range(NCORES)), trace=trace)
    if trace and res.exec_time_ns is not None:
        print(f"HW exec time: {res.exec_time_ns} ns")
    kernel.last_result = res
    og_full = np.stack([r["og"] for r in res.results]).reshape(B * T, N, D).reshape(B, T, N, D)
    oc_full = (
        np.stack([r["oc"] for r in res.results])
        .reshape(B * N, T, D)
        .reshape(B, N, T, D)
        .transpose(0, 2, 1, 3)
    )
    return (og_full + oc_full).astype(np.float32)
